# revision 1
# baseline (speedup 1.0000x reference)
"""Causal self-attention on 8 TRN2 NeuronCores.

Sharding: core c handles batch b = c//2 and head-group hg = c%2 (8 of 16
heads).  Wq/Wk/Wv are split column-wise (per head-group), Wp row-wise; the
row-parallel partial outputs of the two cores sharing a batch are summed on
the host (cheap 8MB adds) together with the bp bias.

Per-core kernel (Tile framework, fp32 PSUM accum everywhere):
  phase A: Q^T, K^T (head-dim on partitions) and V (seq on partitions).
           x and Wq/Wk/Wv arrive as 32x-scaled hi(e4m3)+lo(e5m2) pairs;
           the 3 significant cross terms run as fp8 DoubleRow matmuls
           (0.5 cycles/row, 25% cheaper than bf16 at bf16 precision).
           V carries 64 ones-columns per head so the PV matmul emits the
           softmax denominator pre-broadcast across 64 PSUM rows.
  phase B: per (head, q-chunk): scores^T = K Q^T (bf16) -> one exp per
           two-bank score pair (scale 2^-10 undoes the 32x32 operand
           scaling; flash-style, no max subtraction: scores ~ N(0,1)) ->
           causal mask -> out^T accum = [V|32]^T p^T, PV software-
           pipelined 2 steps behind the scores -> normalize via
           reciprocal+multiply (no broadcast DMA needed).
  phase C: out_partial = attn_out^T.T @ Wp_slice (bf16, row-parallel),
           stored as bf16 and summed on the host.

Scheduling: the attention inner loop is ACT(exp)-bound, so projection
matmuls (QKV of the next chunk, output projections of finished chunks) are
interleaved into the attention emission via filler generators, keeping the
PE fed.  Output projections are deferred to the last (deepest, most
ACT-bound) chunks; the final one pre-starts its ib=0..2 accumulations in
borrowed PSUM banks to overlap the last head's normalize.  Dummy warmup
matmuls during the startup DMAs hold the PE p-state at full clock.  The
1/sqrt(HD) score scale is folded into Wk on the host.
"""

import sys

if "/opt/trn_rl_repo" not in sys.path:
    sys.path.insert(0, "/opt/trn_rl_repo")

from collections import deque
from contextlib import ExitStack

import ml_dtypes
import numpy as np

import concourse.bass as bass
import concourse.tile as tile
from concourse import mybir

P = 128
B, S, D, H = 4, 2048, 1024, 16
HD = 64          # head dim
HPC = 8          # heads per core
DHC = HPC * HD   # 512 inner dims per core
N_CORES = 8
QC = 512         # q-chunk width in phase B
FP32 = mybir.dt.float32
BF = mybir.dt.bfloat16
NPBF = ml_dtypes.bfloat16
NPE4 = ml_dtypes.float8_e4m3
NPE5 = ml_dtypes.float8_e5m2


def split_excess_waits(nc, max_waits=1):
    """walrus TPB_CTRL codegen in this container only accepts 1 sync-wait
    per instruction; hoist extras onto NoOps in front."""
    n = 0
    for fn in nc.m.functions:
        for bb in fn.blocks:
            il = bb.instructions
            i = 0
            while i < len(il):
                ins = il[i]
                si = getattr(ins, "sync_info", None)
                if si is not None and len(si.on_wait) > max_waits:
                    waits = list(si.on_wait)
                    keep = waits[-max_waits:]
                    extra = waits[:-max_waits]
                    for j in range(0, len(extra), max_waits):
                        nop = mybir.InstNoOp(
                            name=f"{ins.name}-wsplit{j}",
                            ins=[],
                            outs=[],
                            engine=ins.engine,
                            sync_info=mybir.SyncInfo(
                                on_wait=extra[j : j + max_waits], on_update=[]
                            ),
                        )
                        il.insert(i, nop)
                        i += 1
                        n += 1
                    si.on_wait = keep
                i += 1
    return n


def pbcast(ap, n):
    """View `ap` ([1, F]) broadcast to n partitions (partition step 0)."""
    return bass.AP(tensor=ap.tensor, offset=ap.offset, ap=[[0, n]] + list(ap.ap[1:]))


def build_attention(nc, io, seq=S):
    Exp = mybir.ActivationFunctionType.Exp
    NQC = seq // QC      # q chunks (also the pipeline step)
    KPQ = QC // P        # k-blocks per q chunk

    DR = mybir.MatmulPerfMode.DoubleRow
    # x and Wq/Wk/Wv come as hi (e4m3) + lo (e5m2) pairs of 32x-scaled
    # values (split on host); the 3 significant cross terms of
    # (xh+xl)@(Wh+Wl) run as DoubleRow matmuls at 0.5 cycles/row, 25%
    # cheaper than bf16 at bf16-level precision.  The 32x32 scale is
    # compensated by the exp's scale immediate (Q,K) and by setting the
    # denominator ones-columns to 32 (V).
    xTh = io["xth"].rearrange("p (c k x) -> p c k x", k=8, x=QC)
    xTl = io["xtl"].rearrange("p (c k x) -> p c k x", k=8, x=QC)
    wqkv = {
        nm: io[nm].rearrange("p (k n) -> p k n", k=8)           # [128, 8, 512]
        for nm in ("wqh", "wql", "wkh", "wkl", "wvh", "wvl")
    }
    wph = io["wph"].rearrange("p (k n) -> p k n", k=4)          # [128, 4, 1024]
    wpl = io["wpl"].rearrange("p (k n) -> p k n", k=4)
    out = io["out"]                                             # [S, 1024]

    with ExitStack() as ctx:
        tc = ctx.enter_context(tile.TileContext(nc))
        const = ctx.enter_context(tc.tile_pool(name="const", bufs=1))
        big = ctx.enter_context(tc.tile_pool(name="big", bufs=1))

        with (
            tc.tile_pool(name="wqkv", bufs=1) as wpool,
            tc.tile_pool(name="xchunk", bufs=2) as xpool,
            tc.tile_pool(name="qtj", bufs=2) as qpool,
            tc.tile_pool(name="pt", bufs=6) as ppool,
            tc.tile_pool(name="small", bufs=3) as spool,
            tc.tile_pool(name="outp", bufs=6) as opool,
            tc.tile_pool(name="psA", bufs=2, space="PSUM") as psA,
            tc.tile_pool(name="psS", bufs=2, space="PSUM") as psS,
            tc.tile_pool(name="psO", bufs=2, space="PSUM") as psO,
        ):
            # ---- startup DMAs, most-urgent first (DMA engines serialize;
            # each sync-queue DMA also pays ~625ns of serialized HWDGE prep,
            # so small constants go on the gpsimd/SWDGE queue instead).
            # Order matches the chunk-0 projection term order: (Wh,xh) first.
            E4 = mybir.dt.float8e4
            E5 = mybir.dt.float8e5
            w_t = {}
            xhs = {0: xpool.tile([P, 8, QC], E4, tag="xth", name="xth0")}
            xls = {0: xpool.tile([P, 8, QC], E5, tag="xtl", name="xtl0")}
            for nm, dt_, src in (
                ("wqh", E4, None), ("xh0", None, None), ("wql", E5, None),
                ("xl0", None, None), ("wkh", E4, None), ("wkl", E5, None),
                ("wvh", E4, None), ("wvl", E5, None),
            ):
                if nm == "xh0":
                    nc.sync.dma_start(xhs[0], xTh[:, 0])
                elif nm == "xl0":
                    nc.sync.dma_start(xls[0], xTl[:, 0])
                else:
                    w_t[nm] = wpool.tile([P, 8, DHC], dt_, name=nm)
                    nc.sync.dma_start(w_t[nm], wqkv[nm])
            wph_t = const.tile([P, 4, 1024], E4)
            nc.sync.dma_start(wph_t, wph)  # first needed by outproj(0), late
            wpl_t = const.tile([P, 4, 1024], E5)
            nc.sync.dma_start(wpl_t, wpl)
            # p-state warmup: the cost of a matmul ramps down only after ~3us
            # of continuous PE busy.  Run dummy matmuls on a zeroed tile while
            # the first wq/xt DMAs are in flight so the real projections start
            # at full clock with no ramp (and no >100ns issue gap to reset it).
            # The memset goes on Pool AHEAD of its const DMAs so the first
            # dummy can issue ~1us in.
            warm = const.tile([P, QC], BF)
            nc.gpsimd.memset(warm, 0.0)
            pswarm = psA.tile([P, QC], FP32, tag="psa", name="pswarm")
            for _ in range(12):
                nc.tensor.matmul(
                    pswarm,
                    lhsT=warm[:, 0:P],
                    rhs=warm,
                    start=True,
                    stop=True,
                    skip_group_check=True,
                )

            bq_t = const.tile([P, 4], FP32)
            nc.gpsimd.dma_start(bq_t, io["bq"])
            bk_t = const.tile([P, 4], FP32)
            nc.gpsimd.dma_start(bk_t, io["bk"])
            bv_t = const.tile([P, DHC], FP32)
            nc.gpsimd.dma_start(bv_t, io["bv"])
            mk_t = const.tile([P, P], BF)          # lower-triangular diag mask
            nc.gpsimd.dma_start(mk_t, io["masks"])

            # persistent per-chunk K^T, V and attn-out tiles.  V is augmented
            # with 64 ones-columns per head so the PV matmul emits the softmax
            # denominator already broadcast across 64 PSUM rows (matmul cost
            # only depends on the moving free size, so the extra columns are
            # free) — normalize needs no partition-broadcast DMA.
            kTs = [big.tile([P, 4, QC], BF, name=f"kT{c}") for c in range(NQC)]
            vAs = [
                big.tile([P, KPQ, HPC, 2 * HD], BF, name=f"vA{c}") for c in range(NQC)
            ]
            aThs = [big.tile([P, 4, QC], mybir.dt.float8e4, name=f"aTh{c}")
                    for c in range(NQC)]
            aTls = [big.tile([P, 4, QC], mybir.dt.float8e5, name=f"aTl{c}")
                    for c in range(NQC)]
            for c in range(NQC):
                nc.gpsimd.memset(vAs[c][:, :, :, HD : 2 * HD], 1.0)

            qT_of = {}

            def qkv_gen(j):
                """QKV projections for chunk j (3-term hi/lo fp8 DoubleRow);
                yields after each instruction so it can be pumped as PE
                filler during attention."""
                xh = xhs.pop(j)
                xl = xls.pop(j)
                qT = qpool.tile([P, 4, QC], BF, tag="qtj")
                qT_of[j] = qT

                def qk(wh, wl, ob, dest, bias, lo_last=False):
                    ps = psA.tile([P, QC], FP32, tag="psa", name="psqk")
                    n = 0
                    # lo_last: chunk-0 K runs before wkl's DMA lands, so do
                    # the (wh, xl) term (operands already resident) first
                    terms = (
                        ((wh, xh), (wh, xl), (wl, xh))
                        if lo_last
                        else ((wh, xh), (wl, xh), (wh, xl))
                    )
                    for W, X in terms:
                        for kp in range(4):
                            nc.tensor.matmul(
                                ps,
                                lhsT=W[:, 2 * kp : 2 * kp + 2, ob * P : (ob + 1) * P],
                                rhs=X[:, 2 * kp : 2 * kp + 2, :],
                                start=(n == 0),
                                stop=(n == 11),
                                perf_mode=DR,
                                skip_group_check=True,
                            )
                            n += 1
                            yield
                    nc.vector.tensor_scalar_add(dest[:, ob, :], ps, bias[:, ob : ob + 1])
                    yield

                def v(sb):
                    psv = psA.tile([P, DHC], FP32, tag="psa", name="psv")
                    n = 0
                    for X, W in (
                        (xh, w_t["wvh"]), (xh, w_t["wvl"]), (xl, w_t["wvh"]),
                    ):
                        for kp in range(4):
                            nc.tensor.matmul(
                                psv,
                                lhsT=X[:, 2 * kp : 2 * kp + 2, sb * P : (sb + 1) * P],
                                rhs=W[:, 2 * kp : 2 * kp + 2, :],
                                start=(n == 0),
                                stop=(n == 11),
                                perf_mode=DR,
                                skip_group_check=True,
                            )
                            n += 1
                            yield
                    nc.vector.tensor_add(
                        vAs[j][:, sb, :, 0:HD],
                        psv.rearrange("p (h d) -> p h d", d=HD),
                        bv_t.rearrange("p (h d) -> p h d", d=HD),
                    )
                    yield

                def q(ob):
                    yield from qk(w_t["wqh"], w_t["wql"], ob, qT, bq_t)

                def k(ob):
                    yield from qk(
                        w_t["wkh"], w_t["wkl"], ob, kTs[j], bk_t, lo_last=(j == 0)
                    )

                if j == 0:
                    # chunk 0 is DMA-limited: consume tensors in arrival
                    # order (wqh, xh, wql, xl, wk*, wv*)
                    for ob in range(4):
                        yield from q(ob)
                    for ob in range(4):
                        yield from k(ob)
                    for sb in range(KPQ):
                        yield from v(sb)
                else:
                    # ob=0 of Q/K plus all of V first: that unblocks head 0 of
                    # the chunk's attention as early as possible.
                    yield from q(0)
                    yield from k(0)
                    for sb in range(KPQ):
                        yield from v(sb)
                    for ob in range(1, 4):
                        yield from q(ob)
                        yield from k(ob)

            def emit_out_store(j, sb, pp0, pp1, on_act, on_swdge=False):
                """Copy an nh pair of PSUM projection tiles into one row tile
                and store it as a single [128, 1024] DMA (fewer HWDGE preps)."""
                ot = opool.tile([P, 1024], BF, tag="ot", name="ot")
                if on_act:
                    nc.scalar.copy(out=ot[:, 0:512], in_=pp0)
                    nc.scalar.copy(out=ot[:, 512:1024], in_=pp1)
                else:
                    nc.vector.tensor_copy(out=ot[:, 0:512], in_=pp0)
                    nc.vector.tensor_copy(out=ot[:, 512:1024], in_=pp1)
                eng = nc.gpsimd if on_swdge else nc.sync
                eng.dma_start(
                    out[(j * KPQ + sb) * P : (j * KPQ + sb + 1) * P, :], ot
                )

            def outproj_gen(j):
                """Row-parallel output projection of chunk j's attn output."""
                for sb in range(KPQ):
                    pps = []
                    for nh in range(2):
                        pp = psA.tile([P, 512], FP32, tag="psa")
                        pps.append(pp)
                        n = 0
                        for A, W in (
                            (aThs[j], wph_t), (aTls[j], wph_t), (aThs[j], wpl_t),
                        ):
                            for ip in range(2):
                                nc.tensor.matmul(
                                    pp,
                                    lhsT=A[:, 2 * ip : 2 * ip + 2,
                                           sb * P : (sb + 1) * P],
                                    rhs=W[:, 2 * ip : 2 * ip + 2,
                                          nh * 512 : (nh + 1) * 512],
                                    start=(n == 0),
                                    stop=(n == 5),
                                    perf_mode=DR,
                                    skip_group_check=True,
                                )
                                n += 1
                                yield
                    emit_out_store(j, sb, pps[0], pps[1], on_act=False)
                    yield

            fillers = deque()

            def pump(n=1):
                while n > 0 and fillers:
                    try:
                        next(fillers[0])
                        n -= 1
                    except StopIteration:
                        fillers.popleft()

            def pump_gen(g):
                """Drain everything up to and including generator g."""
                while fillers and g in fillers:
                    try:
                        next(fillers[0])
                    except StopIteration:
                        fillers.popleft()

            # chunk 0's projections run inline (nothing to overlap yet)
            g0 = qkv_gen(0)
            fillers.append(g0)
            pump_gen(g0)

            for j in range(NQC):
                if j + 1 < NQC:
                    xhs[j + 1] = xpool.tile(
                        [P, 8, QC], E4, tag="xth", name=f"xth{j + 1}"
                    )
                    nc.sync.dma_start(xhs[j + 1], xTh[:, j + 1])
                    xls[j + 1] = xpool.tile(
                        [P, 8, QC], E5, tag="xtl", name=f"xtl{j + 1}"
                    )
                    nc.sync.dma_start(xls[j + 1], xTl[:, j + 1])
                    gnext = qkv_gen(j + 1)
                    fillers.appendleft(gnext)

                nk = KPQ * (j + 1)
                qT = qT_of[j]
                # Attention, software-pipelined by `lag` steps.  k-blocks are
                # processed in PAIRS: both scores land in one two-bank PSUM
                # tile and a single exp covers both (halving the per-
                # instruction ACT overhead — the exp stream is the local
                # bottleneck).  Each step emits its scores+exp+masks, then the
                # PV matmuls from `lag` steps earlier, so the PE always has
                # scores to run while an exp is in flight, including across
                # head boundaries.  pending holds (pv_fn, norm_fn) tuples;
                # norm_fn (the head's normalize) rides with its last PV.
                lag = 2 if j > 0 else 1
                pending = deque()

                def flush_one():
                    pv_fn, norm_fn = pending.popleft()
                    pv_fn()
                    if norm_fn is not None:
                        norm_fn()

                for h in range(HPC):
                    hb, ho = h // 2, (h % 2) * HD
                    po = psO.tile([P, QC], FP32, tag="po")
                    for pi in range(nk // 2):
                        ki0, ki1 = 2 * pi, 2 * pi + 1
                        t0 = ki0 - KPQ * j
                        # first valid q column per k-block (exact causal)
                        off0 = max(t0, 0) * P
                        off1 = max(t0 + 1, 0) * P
                        pair = psS.tile([P, 2, QC], FP32, tag="ps")
                        for i, (ki, off) in enumerate(((ki0, off0), (ki1, off1))):
                            kc, kb = divmod(ki, KPQ)
                            nc.tensor.matmul(
                                pair[:, i, off:],
                                lhsT=kTs[kc][ho : ho + HD, hb, kb * P : (kb + 1) * P],
                                rhs=qT[ho : ho + HD, hb, off:],
                                start=True,
                                stop=True,
                            )
                        ptp = ppool.tile([P, 2, QC], BF, tag="pt")
                        # one exp for both banks; the [off0:off1) slice of
                        # bank 1 is stale-score garbage that is exp'd but
                        # never read (PV/mask slice around it)
                        # scale undoes the 32x on both q and k (32*32=2^10)
                        nc.scalar.activation(
                            ptp[:, :, off0:], pair[:, :, off0:], Exp,
                            scale=2.0 ** -10,
                        )
                        if t0 >= 0:
                            nc.vector.tensor_mul(
                                ptp[:, 0, off0 : off0 + P],
                                ptp[:, 0, off0 : off0 + P],
                                mk_t,
                            )
                            nc.vector.tensor_mul(
                                ptp[:, 1, off1 : off1 + P],
                                ptp[:, 1, off1 : off1 + P],
                                mk_t,
                            )
                        if len(pending) >= lag:
                            flush_one()

                        def pv_fn(po=po, ki0=ki0, ki1=ki1, off0=off0,
                                  off1=off1, ptp=ptp, h=h, nk=nk):
                            for i, (ki, off) in enumerate(
                                ((ki0, off0), (ki1, off1))
                            ):
                                kc, kb = divmod(ki, KPQ)
                                nc.tensor.matmul(
                                    po[:, off:],
                                    lhsT=vAs[kc][:, kb, h, :],
                                    rhs=ptp[:, i, off:],
                                    start=(ki == 0),
                                    stop=(ki == nk - 1),
                                    skip_group_check=True,
                                )
                        norm_fn = None
                        if pi == nk // 2 - 1:
                            def norm_fn(j=j, h=h, hb=hb, ho=ho, po=po):
                                rr = spool.tile([HD, QC], FP32, tag="rr",
                                                name="rr")
                                nc.vector.reciprocal(rr, po[HD : 2 * HD, :])
                                # attn out at 32x (V is 32x, ones-cols are 1):
                                # split into e4m3 hi + e5m2 lo for the DR
                                # output projection
                                # full-height scratch: walrus requires equal
                                # SBUF base partitions for 2-input DVE/Pool
                                # ops, so slice sa at the head's offset
                                sa = spool.tile([P, QC], BF, tag="sa",
                                                name="sa")
                                sah = sa[ho : ho + HD, :]
                                nc.vector.tensor_mul(sah, po[0:HD, :], rr)
                                # hi/lo split on the (otherwise idle) Pool
                                # engine so the DVE stream stays clear for
                                # masks during short-head chunks; later
                                # chunks' long heads have DVE slack, and the
                                # faster DVE ops shorten the last head's
                                # chain into the final projection
                                eng = nc.gpsimd if j < 2 else nc.vector
                                ah = aThs[j][ho : ho + HD, hb, :]
                                eng.tensor_copy(out=ah, in_=sah)
                                eng.tensor_sub(
                                    aTls[j][ho : ho + HD, hb, :], sah, ah
                                )
                        pending.append((pv_fn, norm_fn))
                        # two fillers per pair keeps the PE ahead of the
                        # exp stream (not in chunk 0, whose fillers wait on
                        # the xt(1) DMA)
                        pump(2 if j > 0 else 1)

                while pending:
                    flush_one()
                del qT_of[j]

                if j + 1 < NQC:
                    fillers.append(outproj_gen(j))
                    pump_gen(gnext)
                else:
                    # last chunk: drain leftovers, then run the final output
                    # projection with the ib=0..2 accumulations pre-started in
                    # borrowed PSUM banks so the PE overlaps the last head's
                    # normalize chain; ib=3 (heads 6/7) finishes after it.
                    # (a partial drain here deadlocks: held-back outproj
                    # generators' psA tiles circularly wait on the borrowed
                    # final-projection banks)
                    while fillers:
                        pump(1)
                    groups = [(sb, nh) for sb in range(KPQ) for nh in range(2)]
                    TERMS = ((aThs[j], wph_t), (aTls[j], wph_t), (aThs[j], wpl_t))
                    pps = []
                    fs = None
                    for g, (sb, nh) in enumerate(groups):
                        if g < 4:
                            if g % 2 == 0:
                                fs = psS.tile(
                                    [P, 2, 512], FP32, tag="ps", name=f"fppS{g // 2}"
                                )
                            pp = fs[:, g % 2, :]
                        elif g < 6:
                            pp = psA.tile([P, 512], FP32, tag="psa", name=f"fpp{g}")
                        else:
                            pp = psO.tile([P, QC], FP32, tag="po", name=f"fpp{g}")
                        pps.append(pp)
                        # ib pair 0 (heads 0-3) prestarts while the last
                        # heads' normalize chains complete
                        for t, (A, W) in enumerate(TERMS):
                            nc.tensor.matmul(
                                pp,
                                lhsT=A[:, 0:2, sb * P : (sb + 1) * P],
                                rhs=W[:, 0:2, nh * 512 : (nh + 1) * 512],
                                start=(t == 0),
                                stop=False,
                                perf_mode=DR,
                                skip_group_check=True,
                            )
                    for g, (sb, nh) in enumerate(groups):
                        for t, (A, W) in enumerate(TERMS):
                            nc.tensor.matmul(
                                pps[g],
                                lhsT=A[:, 2:4, sb * P : (sb + 1) * P],
                                rhs=W[:, 2:4, nh * 512 : (nh + 1) * 512],
                                start=False,
                                stop=(t == 2),
                                perf_mode=DR,
                                skip_group_check=True,
                            )
                        if g % 2 == 1:
                            emit_out_store(
                                j, sb, pps[g - 1], pps[g],
                                on_act=(g % 4 == 3), on_swdge=(g % 4 == 3),
                            )

            while fillers:
                pump(1)


def build_program(seq=S, split=True):
    nc = bass.Bass("TRN2", target_bir_lowering=False, debug=False, num_devices=N_CORES)
    E4 = mybir.dt.float8e4
    E5 = mybir.dt.float8e5
    io = {
        "xth": nc.dram_tensor("xth", [P, (seq // QC) * 8 * QC], E4, kind="ExternalInput").ap(),
        "xtl": nc.dram_tensor("xtl", [P, (seq // QC) * 8 * QC], E5, kind="ExternalInput").ap(),
        "wqh": nc.dram_tensor("wqh", [P, 8 * DHC], E4, kind="ExternalInput").ap(),
        "wql": nc.dram_tensor("wql", [P, 8 * DHC], E5, kind="ExternalInput").ap(),
        "wkh": nc.dram_tensor("wkh", [P, 8 * DHC], E4, kind="ExternalInput").ap(),
        "wkl": nc.dram_tensor("wkl", [P, 8 * DHC], E5, kind="ExternalInput").ap(),
        "wvh": nc.dram_tensor("wvh", [P, 8 * DHC], E4, kind="ExternalInput").ap(),
        "wvl": nc.dram_tensor("wvl", [P, 8 * DHC], E5, kind="ExternalInput").ap(),
        "wph": nc.dram_tensor("wph", [P, 4 * D], E4, kind="ExternalInput").ap(),
        "wpl": nc.dram_tensor("wpl", [P, 4 * D], E5, kind="ExternalInput").ap(),
        "bq": nc.dram_tensor("bq", [P, 4], FP32, kind="ExternalInput").ap(),
        "bk": nc.dram_tensor("bk", [P, 4], FP32, kind="ExternalInput").ap(),
        "bv": nc.dram_tensor("bv", [P, DHC], FP32, kind="ExternalInput").ap(),
        "masks": nc.dram_tensor("masks", [P, P], BF, kind="ExternalInput").ap(),
        "out": nc.dram_tensor("out", [seq, D], BF, kind="ExternalOutput").ap(),
    }
    build_attention(nc, io, seq=seq)
    if split:
        split_excess_waits(nc)
    return nc


def make_masks():
    kk = np.arange(P)[:, None]
    qq = np.arange(P)[None, :]
    return np.ascontiguousarray((kk <= qq).astype(NPBF))


def blk_w(w):
    """(K, N) -> [128, (K//128)*N] with row ki holding all (ko, n) blocks."""
    k, n = w.shape
    return np.ascontiguousarray(
        w.reshape(k // P, P, n).transpose(1, 0, 2).reshape(P, (k // P) * n)
    )


def blk_x(xb):
    """x (S, D) -> chunk-major blocked x^T: [128, NQC*8*QC]."""
    seq = xb.shape[0]
    a = xb.T.reshape(8, P, seq)  # [ko, ki, s]
    b = a.transpose(1, 0, 2).reshape(P, 8, seq // QC, QC).transpose(0, 2, 1, 3)
    return np.ascontiguousarray(b.reshape(P, (seq // QC) * 8 * QC))


def hilo(a):
    """Split 32x-scaled values into an e4m3 hi + e5m2 lo residual pair."""
    hi = a.astype(NPE4)
    lo = (a - hi.astype(np.float32)).astype(NPE5)
    return hi, lo


def shard_inputs(x, Wq, bq, Wk, bk, Wv, bv, Wp, bp, seq=S):
    masks = make_masks()
    in_maps = []
    xcache = {}
    for c in range(N_CORES):
        b, hg = c // 2, c % 2
        cols = slice(hg * DHC, (hg + 1) * DHC)
        # biases carry the same 32x scale as their weights
        bqc = np.ascontiguousarray((bq[cols] * 32.0).reshape(4, P).T)
        bkc = np.ascontiguousarray((bk[cols] * 4.0).reshape(4, P).T)
        bvc = np.ascontiguousarray(np.tile(bv[cols][None, :] * 32.0, (P, 1)))
        if b not in xcache:
            xcache[b] = hilo(blk_x(x[b]))
        xh, xl = xcache[b]
        wqh, wql = hilo(blk_w(Wq[:, cols] * 32.0))
        wkh, wkl = hilo(blk_w(Wk[:, cols] * 4.0))   # 32 * 1/8 score scale
        wvh, wvl = hilo(blk_w(Wv[:, cols] * 32.0))
        wph, wpl = hilo(blk_w(Wp[cols, :] * 32.0))
        in_maps.append(
            {
                "xth": xh,
                "xtl": xl,
                "wqh": wqh,
                "wql": wql,
                "wkh": wkh,
                "wkl": wkl,
                "wvh": wvh,
                "wvl": wvl,
                "wph": wph,
                "wpl": wpl,
                "bq": bqc,
                "bk": bkc,
                "bv": bvc,
                "masks": masks,
            }
        )
    return in_maps


_NC_CACHE = {}


def _get_nc(seq=S):
    if seq not in _NC_CACHE:
        _NC_CACHE[seq] = build_program(seq)
    return _NC_CACHE[seq]


def kernel(x, Wq, bq, Wk, bk, Wv, bv, Wp, bp, **run_kwargs):
    from concourse.bass_utils import run_bass_kernel_spmd

    x = np.asarray(x, np.float32)
    Wq, Wk, Wv, Wp = (np.asarray(a, np.float32) for a in (Wq, Wk, Wv, Wp))
    bq, bk, bv, bp = (np.asarray(a, np.float32) for a in (bq, bk, bv, bp))

    nc = _get_nc()
    in_maps = shard_inputs(x, Wq, bq, Wk, bk, Wv, bv, Wp, bp)
    res = run_bass_kernel_spmd(nc, in_maps, core_ids=list(range(N_CORES)), **run_kwargs)
    parts = [np.asarray(res.results[c]["out"], np.float32) for c in range(N_CORES)]
    out = np.empty((B, S, D), np.float32)
    for b in range(B):
        # partials carry the 32x(a) * 32x(Wp) scale; undo it here
        out[b] = (parts[2 * b] + parts[2 * b + 1]) * 2.0**-10 + bp
    kernel.last_results = res
    return out



# revision 5
# speedup vs baseline: 1.1730x; 1.1730x over previous
"""Causal self-attention on 8 TRN2 NeuronCores.

Sharding: core c handles batch b = c//2 and head-group hg = c%2 (8 of 16
heads).  Wq/Wk/Wv are split column-wise (per head-group), Wp row-wise; the
row-parallel partial outputs of the two cores sharing a batch are summed on
the host (cheap 8MB adds) together with the bp bias.

Per-core kernel (Tile framework, fp32 PSUM accum everywhere):
  phase A: Q^T, K^T (head-dim on partitions) and V (seq on partitions),
           plain bf16 matmuls (on HW, fp8 DoubleRow streams moving columns
           no faster than bf16, so a single bf16 term beats 3-term hi/lo
           fp8 by 1.5x).  V carries 64 ones-columns per head so the PV
           matmul emits the softmax denominator pre-broadcast across 64
           PSUM rows (matmul cost only depends on the moving free size, so
           the extra columns are free).
  phase B: per (head, q-chunk): scores^T = K Q^T (bf16) -> one exp per
           two-bank score pair (scale 1/8 applies the 1/sqrt(HD) score
           scale; flash-style, no max subtraction: scores ~ N(0,1)) ->
           causal mask (Pool engine; DVE stays clear) -> out^T accum =
           [V|1]^T p^T, PV software-pipelined 2 steps behind the scores ->
           normalize via ACT-engine reciprocal + DVE multiply straight
           into the bf16 attn-out tile.
  phase C: out_partial = attn_out^T.T @ Wp_slice (bf16, row-parallel),
           stored as bf16 and summed on the host.

Scheduling: projection matmuls (QKV of the next chunk, output projections
of finished chunks) are interleaved into the attention emission via filler
generators, keeping the PE fed.  Output projections are deferred to the
last (deepest) chunks; the final one pre-starts its ib=0..1 accumulations
in borrowed PSUM banks to overlap the last head's normalize.  Dummy warmup
matmuls during the startup DMAs hold the PE p-state at full clock.
"""

import sys

if "/opt/trn_rl_repo" not in sys.path:
    sys.path.insert(0, "/opt/trn_rl_repo")

from collections import deque
from contextlib import ExitStack

import ml_dtypes
import numpy as np

import concourse.bass as bass
import concourse.tile as tile
from concourse import mybir

P = 128
B, S, D, H = 4, 2048, 1024, 16
HD = 64          # head dim
HPC = 8          # heads per core
DHC = HPC * HD   # 512 inner dims per core
N_CORES = 8
QC = 512         # q-chunk width in phase B
FP32 = mybir.dt.float32
BF = mybir.dt.bfloat16
NPBF = ml_dtypes.bfloat16


def split_excess_waits(nc, max_waits=1):
    """walrus TPB_CTRL codegen in this container only accepts 1 sync-wait
    per instruction; hoist extras onto NoOps in front."""
    n = 0
    for fn in nc.m.functions:
        for bb in fn.blocks:
            il = bb.instructions
            i = 0
            while i < len(il):
                ins = il[i]
                si = getattr(ins, "sync_info", None)
                if si is not None and len(si.on_wait) > max_waits:
                    waits = list(si.on_wait)
                    keep = waits[-max_waits:]
                    extra = waits[:-max_waits]
                    for j in range(0, len(extra), max_waits):
                        nop = mybir.InstNoOp(
                            name=f"{ins.name}-wsplit{j}",
                            ins=[],
                            outs=[],
                            engine=ins.engine,
                            sync_info=mybir.SyncInfo(
                                on_wait=extra[j : j + max_waits], on_update=[]
                            ),
                        )
                        il.insert(i, nop)
                        i += 1
                        n += 1
                    si.on_wait = keep
                i += 1
    return n


def build_attention(nc, io, seq=S):
    Exp = mybir.ActivationFunctionType.Exp
    NQC = seq // QC      # q chunks (also the pipeline step)
    KPQ = QC // P        # k-blocks per q chunk

    xT = io["xt"].rearrange("p (c k x) -> p c k x", k=8, x=QC)
    wqkv = {
        nm: io[nm].rearrange("p (k n) -> p k n", k=8)           # [128, 8, 512]
        for nm in ("wq", "wk", "wv")
    }
    wp = io["wp"].rearrange("p (k n) -> p k n", k=4)            # [128, 4, 1024]
    out = io["out"]                                             # [S, 1024]

    with ExitStack() as ctx:
        tc = ctx.enter_context(tile.TileContext(nc))
        const = ctx.enter_context(tc.tile_pool(name="const", bufs=1))
        big = ctx.enter_context(tc.tile_pool(name="big", bufs=1))

        with (
            tc.tile_pool(name="wqkv", bufs=1) as wpool,
            tc.tile_pool(name="xchunk", bufs=2) as xpool,
            tc.tile_pool(name="qtj", bufs=2) as qpool,
            tc.tile_pool(name="pt", bufs=6) as ppool,
            tc.tile_pool(name="small", bufs=3) as spool,
            tc.tile_pool(name="outp", bufs=6) as opool,
            tc.tile_pool(name="psA", bufs=2, space="PSUM") as psA,
            tc.tile_pool(name="psS", bufs=2, space="PSUM") as psS,
            tc.tile_pool(name="psO", bufs=2, space="PSUM") as psO,
        ):
            # ---- startup DMAs, most-urgent first (DMA engines serialize;
            # each sync-queue DMA also pays ~625ns of serialized HWDGE prep,
            # so small constants go on the gpsimd/SWDGE queue instead).
            # Order matches chunk-0 consumption: Q needs wq+x0, then wk, wv.
            w_t = {}
            xs = {0: xpool.tile([P, 8, QC], BF, tag="xt", name="xt0")}
            w_t["wq"] = wpool.tile([P, 8, DHC], BF, name="wq")
            nc.sync.dma_start(w_t["wq"], wqkv["wq"])
            nc.sync.dma_start(xs[0], xT[:, 0])
            w_t["wk"] = wpool.tile([P, 8, DHC], BF, name="wk")
            nc.sync.dma_start(w_t["wk"], wqkv["wk"])
            w_t["wv"] = wpool.tile([P, 8, DHC], BF, name="wv")
            nc.sync.dma_start(w_t["wv"], wqkv["wv"])
            wp_t = const.tile([P, 4, 1024], BF)
            nc.sync.dma_start(wp_t, wp)  # first needed by outproj(0), late
            # p-state warmup: the cost of a matmul ramps down only after ~3us
            # of continuous PE busy.  Run dummy matmuls on a zeroed tile while
            # the first wq/xt DMAs are in flight so the real projections start
            # at full clock with no ramp (and no >100ns issue gap to reset it).
            warm = const.tile([P, QC], BF)
            nc.gpsimd.memset(warm, 0.0)
            pswarm = psA.tile([P, QC], FP32, tag="psa", name="pswarm")
            for _ in range(12):
                nc.tensor.matmul(
                    pswarm,
                    lhsT=warm[:, 0:P],
                    rhs=warm,
                    start=True,
                    stop=True,
                    skip_group_check=True,
                )

            bq_t = const.tile([P, 4], FP32)
            nc.gpsimd.dma_start(bq_t, io["bq"])
            bk_t = const.tile([P, 4], FP32)
            nc.gpsimd.dma_start(bk_t, io["bk"])
            bv_t = const.tile([P, DHC], FP32)
            nc.gpsimd.dma_start(bv_t, io["bv"])
            mk_t = const.tile([P, P], BF)          # lower-triangular diag mask
            nc.gpsimd.dma_start(mk_t, io["masks"])

            # persistent per-chunk K^T, V and attn-out tiles.  V is augmented
            # with 64 ones-columns per head so the PV matmul emits the softmax
            # denominator already broadcast across 64 PSUM rows (matmul cost
            # only depends on the moving free size, so the extra columns are
            # free) — normalize needs no partition-broadcast DMA.
            kTs = [big.tile([P, 4, QC], BF, name=f"kT{c}") for c in range(NQC)]
            vAs = [
                big.tile([P, KPQ, HPC, 2 * HD], BF, name=f"vA{c}") for c in range(NQC)
            ]
            aTs = [big.tile([P, 4, QC], BF, name=f"aT{c}") for c in range(NQC)]
            for c in range(NQC):
                nc.gpsimd.memset(vAs[c][:, :, :, HD : 2 * HD], 1.0)

            qT_of = {}

            def qkv_gen(j):
                """QKV projections for chunk j (single-term bf16); yields
                after each instruction so it can be pumped as PE filler
                during attention."""
                x = xs.pop(j)
                qT = qpool.tile([P, 4, QC], BF, tag="qtj")
                qT_of[j] = qT

                def qk(w, ob, dest, bias):
                    ps = psA.tile([P, QC], FP32, tag="psa", name="psqk")
                    for kp in range(8):
                        nc.tensor.matmul(
                            ps,
                            lhsT=w[:, kp, ob * P : (ob + 1) * P],
                            rhs=x[:, kp, :],
                            start=(kp == 0),
                            stop=(kp == 7),
                            skip_group_check=True,
                        )
                        yield
                    nc.vector.tensor_scalar_add(dest[:, ob, :], ps, bias[:, ob : ob + 1])
                    yield

                def v(sb):
                    psv = psA.tile([P, DHC], FP32, tag="psa", name="psv")
                    for kp in range(8):
                        nc.tensor.matmul(
                            psv,
                            lhsT=x[:, kp, sb * P : (sb + 1) * P],
                            rhs=w_t["wv"][:, kp, :],
                            start=(kp == 0),
                            stop=(kp == 7),
                            skip_group_check=True,
                        )
                        yield
                    nc.vector.tensor_add(
                        vAs[j][:, sb, :, 0:HD],
                        psv.rearrange("p (h d) -> p h d", d=HD),
                        bv_t.rearrange("p (h d) -> p h d", d=HD),
                    )
                    yield

                def q(ob):
                    yield from qk(w_t["wq"], ob, qT, bq_t)

                def k(ob):
                    yield from qk(w_t["wk"], ob, kTs[j], bk_t)

                if j == 0:
                    # chunk 0 is DMA-limited: consume tensors in arrival
                    # order (wq+x, wk, wv)
                    for ob in range(4):
                        yield from q(ob)
                    for ob in range(4):
                        yield from k(ob)
                    for sb in range(KPQ):
                        yield from v(sb)
                else:
                    # ob=0 of Q/K plus all of V first: that unblocks head 0 of
                    # the chunk's attention as early as possible.
                    yield from q(0)
                    yield from k(0)
                    for sb in range(KPQ):
                        yield from v(sb)
                    for ob in range(1, 4):
                        yield from q(ob)
                        yield from k(ob)

            def emit_out_store(j, sb, pp0, pp1, on_act, on_swdge=False):
                """Copy an nh pair of PSUM projection tiles into one row tile
                and store it as a single [128, 1024] DMA (fewer HWDGE preps)."""
                ot = opool.tile([P, 1024], BF, tag="ot", name="ot")
                if on_act:
                    nc.scalar.copy(out=ot[:, 0:512], in_=pp0)
                    nc.scalar.copy(out=ot[:, 512:1024], in_=pp1)
                else:
                    nc.vector.tensor_copy(out=ot[:, 0:512], in_=pp0)
                    nc.vector.tensor_copy(out=ot[:, 512:1024], in_=pp1)
                eng = nc.gpsimd if on_swdge else nc.sync
                eng.dma_start(
                    out[(j * KPQ + sb) * P : (j * KPQ + sb + 1) * P, :], ot
                )

            def outproj_gen(j):
                """Row-parallel output projection of chunk j's attn output."""
                for sb in range(KPQ):
                    pps = []
                    for nh in range(2):
                        pp = psA.tile([P, 512], FP32, tag="psa")
                        pps.append(pp)
                        for ib in range(4):
                            nc.tensor.matmul(
                                pp,
                                lhsT=aTs[j][:, ib, sb * P : (sb + 1) * P],
                                rhs=wp_t[:, ib, nh * 512 : (nh + 1) * 512],
                                start=(ib == 0),
                                stop=(ib == 3),
                                skip_group_check=True,
                            )
                            yield
                    emit_out_store(j, sb, pps[0], pps[1], on_act=False)
                    yield

            fillers = deque()

            def pump(n=1):
                while n > 0 and fillers:
                    try:
                        next(fillers[0])
                        n -= 1
                    except StopIteration:
                        fillers.popleft()

            def pump_gen(g):
                """Drain everything up to and including generator g."""
                while fillers and g in fillers:
                    try:
                        next(fillers[0])
                    except StopIteration:
                        fillers.popleft()

            # chunk 0's projections run inline (nothing to overlap yet)
            g0 = qkv_gen(0)
            fillers.append(g0)
            pump_gen(g0)

            for j in range(NQC):
                if j + 1 < NQC:
                    xs[j + 1] = xpool.tile(
                        [P, 8, QC], BF, tag="xt", name=f"xt{j + 1}"
                    )
                    nc.sync.dma_start(xs[j + 1], xT[:, j + 1])
                    gnext = qkv_gen(j + 1)
                    fillers.appendleft(gnext)

                nk = KPQ * (j + 1)
                qT = qT_of[j]
                # Attention, software-pipelined by `lag` steps.  k-blocks are
                # processed in PAIRS: both scores land in one two-bank PSUM
                # tile and a single exp covers both (halving the per-
                # instruction ACT overhead — the exp stream is the local
                # bottleneck).  Each step emits its scores+exp+masks, then the
                # PV matmuls from `lag` steps earlier, so the PE always has
                # scores to run while an exp is in flight, including across
                # head boundaries.  pending holds (pv_fn, norm_fn) tuples;
                # norm_fn (the head's normalize) rides with its last PV.
                lag = 2 if j > 0 else 1
                pending = deque()

                def flush_one():
                    pv_fn, norm_fn = pending.popleft()
                    pv_fn()
                    if norm_fn is not None:
                        norm_fn()

                for h in range(HPC):
                    hb, ho = h // 2, (h % 2) * HD
                    po = psO.tile([P, QC], FP32, tag="po")
                    for pi in range(nk // 2):
                        ki0, ki1 = 2 * pi, 2 * pi + 1
                        t0 = ki0 - KPQ * j
                        # first valid q column per k-block (exact causal)
                        off0 = max(t0, 0) * P
                        off1 = max(t0 + 1, 0) * P
                        pair = psS.tile([P, 2, QC], FP32, tag="ps")
                        for i, (ki, off) in enumerate(((ki0, off0), (ki1, off1))):
                            kc, kb = divmod(ki, KPQ)
                            nc.tensor.matmul(
                                pair[:, i, off:],
                                lhsT=kTs[kc][ho : ho + HD, hb, kb * P : (kb + 1) * P],
                                rhs=qT[ho : ho + HD, hb, off:],
                                start=True,
                                stop=True,
                            )
                        ptp = ppool.tile([P, 2, QC], BF, tag="pt")
                        # one exp for both banks; the [off0:off1) slice of
                        # bank 1 is stale-score garbage that is exp'd but
                        # never read (PV/mask slice around it).
                        # scale 1/8 applies the 1/sqrt(HD) softmax scale.
                        nc.scalar.activation(
                            ptp[:, :, off0:], pair[:, :, off0:], Exp,
                            scale=2.0 ** -3,
                        )
                        if t0 >= 0:
                            # diag-block triangle mask on the (otherwise
                            # idle) Pool engine; DVE keeps the normalize path
                            nc.gpsimd.tensor_mul(
                                ptp[:, 0, off0 : off0 + P],
                                ptp[:, 0, off0 : off0 + P],
                                mk_t,
                            )
                            nc.gpsimd.tensor_mul(
                                ptp[:, 1, off1 : off1 + P],
                                ptp[:, 1, off1 : off1 + P],
                                mk_t,
                            )
                        if len(pending) >= lag:
                            flush_one()

                        def pv_fn(po=po, ki0=ki0, ki1=ki1, off0=off0,
                                  off1=off1, ptp=ptp, h=h, nk=nk):
                            for i, (ki, off) in enumerate(
                                ((ki0, off0), (ki1, off1))
                            ):
                                kc, kb = divmod(ki, KPQ)
                                nc.tensor.matmul(
                                    po[:, off:],
                                    lhsT=vAs[kc][:, kb, h, :],
                                    rhs=ptp[:, i, off:],
                                    start=(ki == 0),
                                    stop=(ki == nk - 1),
                                    skip_group_check=True,
                                )
                        norm_fn = None
                        if pi == nk // 2 - 1:
                            def norm_fn(j=j, h=h, hb=hb, ho=ho, po=po):
                                # denominator reciprocal (custom-DVE fast
                                # recip doesn't compile in this container),
                                # then one DVE multiply straight into the
                                # bf16 attn-out tile.
                                rr = spool.tile([HD, QC], FP32, tag="rr",
                                                name="rr")
                                nc.vector.reciprocal(rr, po[HD : 2 * HD, :])
                                nc.vector.tensor_mul(
                                    aTs[j][ho : ho + HD, hb, :],
                                    po[0:HD, :],
                                    rr,
                                )
                        pending.append((pv_fn, norm_fn))
                        # two fillers per pair keeps the PE ahead of the
                        # exp stream (not in chunk 0, whose fillers wait on
                        # the xt(1) DMA)
                        pump(2 if j > 0 else 1)

                while pending:
                    flush_one()
                del qT_of[j]

                if j + 1 < NQC:
                    fillers.append(outproj_gen(j))
                    pump_gen(gnext)
                else:
                    # last chunk: drain leftovers, then run the final output
                    # projection with the ib=0..1 accumulations pre-started in
                    # borrowed PSUM banks so the PE overlaps the last head's
                    # normalize chain; ib=2..3 (heads 4-7) finishes after it.
                    # (a partial drain here deadlocks: held-back outproj
                    # generators' psA tiles circularly wait on the borrowed
                    # final-projection banks)
                    while fillers:
                        pump(1)
                    groups = [(sb, nh) for sb in range(KPQ) for nh in range(2)]
                    pps = []
                    fs = None
                    for g, (sb, nh) in enumerate(groups):
                        if g < 4:
                            if g % 2 == 0:
                                fs = psS.tile(
                                    [P, 2, 512], FP32, tag="ps", name=f"fppS{g // 2}"
                                )
                            pp = fs[:, g % 2, :]
                        elif g < 6:
                            pp = psA.tile([P, 512], FP32, tag="psa", name=f"fpp{g}")
                        else:
                            pp = psO.tile([P, QC], FP32, tag="po", name=f"fpp{g}")
                        pps.append(pp)
                        # ib 0..1 prestarts while the last heads' normalize
                        # chains complete
                        for ib in range(2):
                            nc.tensor.matmul(
                                pp,
                                lhsT=aTs[j][:, ib, sb * P : (sb + 1) * P],
                                rhs=wp_t[:, ib, nh * 512 : (nh + 1) * 512],
                                start=(ib == 0),
                                stop=False,
                                skip_group_check=True,
                            )
                    for g, (sb, nh) in enumerate(groups):
                        for ib in range(2, 4):
                            nc.tensor.matmul(
                                pps[g],
                                lhsT=aTs[j][:, ib, sb * P : (sb + 1) * P],
                                rhs=wp_t[:, ib, nh * 512 : (nh + 1) * 512],
                                start=False,
                                stop=(ib == 3),
                                skip_group_check=True,
                            )
                        if g % 2 == 1:
                            emit_out_store(
                                j, sb, pps[g - 1], pps[g],
                                on_act=(g % 4 == 3), on_swdge=(g % 4 == 3),
                            )

            while fillers:
                pump(1)


def build_program(seq=S, split=True):
    nc = bass.Bass("TRN2", target_bir_lowering=False, debug=False, num_devices=N_CORES)
    io = {
        "xt": nc.dram_tensor("xt", [P, (seq // QC) * 8 * QC], BF, kind="ExternalInput").ap(),
        "wq": nc.dram_tensor("wq", [P, 8 * DHC], BF, kind="ExternalInput").ap(),
        "wk": nc.dram_tensor("wk", [P, 8 * DHC], BF, kind="ExternalInput").ap(),
        "wv": nc.dram_tensor("wv", [P, 8 * DHC], BF, kind="ExternalInput").ap(),
        "wp": nc.dram_tensor("wp", [P, 4 * D], BF, kind="ExternalInput").ap(),
        "bq": nc.dram_tensor("bq", [P, 4], FP32, kind="ExternalInput").ap(),
        "bk": nc.dram_tensor("bk", [P, 4], FP32, kind="ExternalInput").ap(),
        "bv": nc.dram_tensor("bv", [P, DHC], FP32, kind="ExternalInput").ap(),
        "masks": nc.dram_tensor("masks", [P, P], BF, kind="ExternalInput").ap(),
        "out": nc.dram_tensor("out", [seq, D], BF, kind="ExternalOutput").ap(),
    }
    build_attention(nc, io, seq=seq)
    if split:
        split_excess_waits(nc)
    return nc


def make_masks():
    kk = np.arange(P)[:, None]
    qq = np.arange(P)[None, :]
    return np.ascontiguousarray((kk <= qq).astype(NPBF))


def blk_w(w):
    """(K, N) -> [128, (K//128)*N] with row ki holding all (ko, n) blocks."""
    k, n = w.shape
    return np.ascontiguousarray(
        w.reshape(k // P, P, n).transpose(1, 0, 2).reshape(P, (k // P) * n)
    )


def blk_x(xb):
    """x (S, D) -> chunk-major blocked x^T: [128, NQC*8*QC]."""
    seq = xb.shape[0]
    a = xb.T.reshape(8, P, seq)  # [ko, ki, s]
    b = a.transpose(1, 0, 2).reshape(P, 8, seq // QC, QC).transpose(0, 2, 1, 3)
    return np.ascontiguousarray(b.reshape(P, (seq // QC) * 8 * QC))


def shard_inputs(x, Wq, bq, Wk, bk, Wv, bv, Wp, bp, seq=S):
    masks = make_masks()
    in_maps = []
    xcache = {}
    for c in range(N_CORES):
        b, hg = c // 2, c % 2
        cols = slice(hg * DHC, (hg + 1) * DHC)
        bqc = np.ascontiguousarray(bq[cols].reshape(4, P).T)
        bkc = np.ascontiguousarray(bk[cols].reshape(4, P).T)
        bvc = np.ascontiguousarray(np.tile(bv[cols][None, :], (P, 1)))
        if b not in xcache:
            xcache[b] = blk_x(x[b]).astype(NPBF)
        in_maps.append(
            {
                "xt": xcache[b],
                "wq": blk_w(Wq[:, cols]).astype(NPBF),
                "wk": blk_w(Wk[:, cols]).astype(NPBF),
                "wv": blk_w(Wv[:, cols]).astype(NPBF),
                "wp": blk_w(Wp[cols, :]).astype(NPBF),
                "bq": bqc,
                "bk": bkc,
                "bv": bvc,
                "masks": masks,
            }
        )
    return in_maps


_NC_CACHE = {}


def _get_nc(seq=S):
    if seq not in _NC_CACHE:
        _NC_CACHE[seq] = build_program(seq)
    return _NC_CACHE[seq]


def kernel(x, Wq, bq, Wk, bk, Wv, bv, Wp, bp, **run_kwargs):
    from concourse.bass_utils import run_bass_kernel_spmd

    x = np.asarray(x, np.float32)
    Wq, Wk, Wv, Wp = (np.asarray(a, np.float32) for a in (Wq, Wk, Wv, Wp))
    bq, bk, bv, bp = (np.asarray(a, np.float32) for a in (bq, bk, bv, bp))

    nc = _get_nc()
    in_maps = shard_inputs(x, Wq, bq, Wk, bk, Wv, bv, Wp, bp)
    res = run_bass_kernel_spmd(nc, in_maps, core_ids=list(range(N_CORES)), **run_kwargs)
    parts = [np.asarray(res.results[c]["out"], np.float32) for c in range(N_CORES)]
    out = np.empty((B, S, D), np.float32)
    for b in range(B):
        out[b] = parts[2 * b] + parts[2 * b + 1] + bp
    kernel.last_results = res
    return out


# revision 10
# speedup vs baseline: 1.2791x; 1.0905x over previous
"""Causal self-attention on 8 TRN2 NeuronCores.

Sharding: core c handles batch b = c//2 and head-group hg = c%2 (8 of 16
heads).  Wq/Wk/Wv are split column-wise (per head-group), Wp row-wise; the
row-parallel partial outputs of the two cores sharing a batch are summed on
the host (cheap 8MB adds) together with the bp bias.

Per-core kernel (Tile framework, fp32 PSUM accum everywhere):
  phase A: Q^T, K^T (head-dim on partitions) and V (seq on partitions),
           plain bf16 matmuls (on HW, fp8 DoubleRow streams moving columns
           no faster than bf16, so a single bf16 term beats 3-term hi/lo
           fp8 by 1.5x).  V carries 64 ones-columns per head so the PV
           matmul emits the softmax denominator pre-broadcast across 64
           PSUM rows (matmul cost only depends on the moving free size, so
           the extra columns are free).
  phase B: per (head, q-chunk): scores^T = K Q^T (bf16) -> one exp per
           two-bank score pair (scale 1/8 applies the 1/sqrt(HD) score
           scale; flash-style, no max subtraction: scores ~ N(0,1)) ->
           causal mask (Pool engine; DVE stays clear) -> out^T accum =
           [V|1]^T p^T, PV software-pipelined 2 steps behind the scores ->
           normalize via ACT-engine reciprocal + DVE multiply straight
           into the bf16 attn-out tile.
  phase C: out_partial = attn_out^T.T @ Wp_slice (bf16, row-parallel),
           stored as bf16 and summed on the host.

Scheduling: projection matmuls (QKV of the next chunk, output projections
of finished chunks) are interleaved into the attention emission via filler
generators, keeping the PE fed.  Output projections are deferred to the
last (deepest) chunks; the final one pre-starts its ib=0..1 accumulations
in borrowed PSUM banks to overlap the last head's normalize.  Dummy warmup
matmuls during the startup DMAs hold the PE p-state at full clock.
"""

import sys

if "/opt/trn_rl_repo" not in sys.path:
    sys.path.insert(0, "/opt/trn_rl_repo")

from collections import deque
from contextlib import ExitStack

import ml_dtypes
import numpy as np

import concourse.bass as bass
import concourse.tile as tile
from concourse import mybir

P = 128
B, S, D, H = 4, 2048, 1024, 16
HD = 64          # head dim
HPC = 8          # heads per core
DHC = HPC * HD   # 512 inner dims per core
N_CORES = 8
QC = 512         # q-chunk width in phase B
FP32 = mybir.dt.float32
BF = mybir.dt.bfloat16
NPBF = ml_dtypes.bfloat16


def split_excess_waits(nc, max_waits=1):
    """walrus TPB_CTRL codegen in this container only accepts 1 sync-wait
    per instruction; hoist extras onto NoOps in front."""
    n = 0
    for fn in nc.m.functions:
        for bb in fn.blocks:
            il = bb.instructions
            i = 0
            while i < len(il):
                ins = il[i]
                si = getattr(ins, "sync_info", None)
                if si is not None and len(si.on_wait) > max_waits:
                    waits = list(si.on_wait)
                    keep = waits[-max_waits:]
                    extra = waits[:-max_waits]
                    for j in range(0, len(extra), max_waits):
                        nop = mybir.InstNoOp(
                            name=f"{ins.name}-wsplit{j}",
                            ins=[],
                            outs=[],
                            engine=ins.engine,
                            sync_info=mybir.SyncInfo(
                                on_wait=extra[j : j + max_waits], on_update=[]
                            ),
                        )
                        il.insert(i, nop)
                        i += 1
                        n += 1
                    si.on_wait = keep
                i += 1
    return n


def build_attention(nc, io, seq=S):
    Exp = mybir.ActivationFunctionType.Exp
    Ln = mybir.ActivationFunctionType.Ln
    NQC = seq // QC      # q chunks (also the pipeline step)
    KPQ = QC // P        # k-blocks per q chunk

    xT = io["xt"].rearrange("p (c k x) -> p c k x", k=8, x=QC)
    wqkv = {
        nm: io[nm].rearrange("p (k n) -> p k n", k=8)           # [128, 8, 512]
        for nm in ("wq", "wk", "wv")
    }
    wp = io["wp"].rearrange("p (k n) -> p k n", k=4)            # [128, 4, 1024]
    out = io["out"]                                             # [S, 1024]

    with ExitStack() as ctx:
        tc = ctx.enter_context(tile.TileContext(nc))
        const = ctx.enter_context(tc.tile_pool(name="const", bufs=1))
        big = ctx.enter_context(tc.tile_pool(name="big", bufs=1))

        with (
            tc.tile_pool(name="wqkv", bufs=1) as wpool,
            tc.tile_pool(name="xchunk", bufs=2) as xpool,
            tc.tile_pool(name="qtj", bufs=2) as qpool,
            tc.tile_pool(name="pt", bufs=6) as ppool,
            tc.tile_pool(name="small", bufs=3) as spool,
            tc.tile_pool(name="outp", bufs=6) as opool,
            tc.tile_pool(name="psA", bufs=2, space="PSUM") as psA,
            tc.tile_pool(name="psS", bufs=2, space="PSUM") as psS,
            tc.tile_pool(name="psO", bufs=2, space="PSUM") as psO,
        ):
            # ---- startup DMAs, most-urgent first (DMA engines serialize;
            # each sync-queue DMA also pays ~625ns of serialized HWDGE prep,
            # so small constants go on the gpsimd/SWDGE queue instead).
            # Order matches chunk-0 consumption: Q needs wq+x0, then wk, wv.
            w_t = {}
            xs = {0: xpool.tile([P, 8, QC], BF, tag="xt", name="xt0")}
            w_t["wq"] = wpool.tile([P, 8, DHC], BF, name="wq")
            nc.sync.dma_start(w_t["wq"], wqkv["wq"])
            # x chunks ride the gpsimd/SWDGE queue so they transfer in
            # parallel with the weights on the serialized sync queue
            nc.gpsimd.dma_start(xs[0], xT[:, 0])
            w_t["wk"] = wpool.tile([P, 8, DHC], BF, name="wk")
            nc.sync.dma_start(w_t["wk"], wqkv["wk"])
            w_t["wv"] = wpool.tile([P, 8, DHC], BF, name="wv")
            nc.sync.dma_start(w_t["wv"], wqkv["wv"])
            wp_t = const.tile([P, 4, 1024], BF)
            nc.sync.dma_start(wp_t, wp)  # first needed by outproj(0), late
            # p-state warmup: the cost of a matmul ramps down only after ~3us
            # of continuous PE busy.  Run dummy matmuls on a zeroed tile while
            # the first wq/xt DMAs are in flight so the real projections start
            # at full clock with no ramp (and no >100ns issue gap to reset it).
            warm = const.tile([P, QC], BF)
            nc.gpsimd.memset(warm, 0.0)
            pswarm = psA.tile([P, QC], FP32, tag="psa", name="pswarm")
            for _ in range(12):
                nc.tensor.matmul(
                    pswarm,
                    lhsT=warm[:, 0:P],
                    rhs=warm,
                    start=True,
                    stop=True,
                    skip_group_check=True,
                )

            bq_t = const.tile([P, 4], FP32)
            nc.gpsimd.dma_start(bq_t, io["bq"])
            bk_t = const.tile([P, 4], FP32)
            nc.gpsimd.dma_start(bk_t, io["bk"])
            bv_t = const.tile([P, DHC], FP32)
            nc.gpsimd.dma_start(bv_t, io["bv"])
            mk_t = const.tile([P, P], BF)          # lower-triangular diag mask
            nc.gpsimd.dma_start(mk_t, io["masks"])

            # persistent per-chunk K^T, V and attn-out tiles.  V is augmented
            # with 64 ones-columns per head so the PV matmul emits the softmax
            # denominator already broadcast across 64 PSUM rows (matmul cost
            # only depends on the moving free size, so the extra columns are
            # free) — normalize needs no partition-broadcast DMA.
            kTs = [big.tile([P, 4, QC], BF, name=f"kT{c}") for c in range(NQC)]
            vAs = [
                big.tile([P, KPQ, HPC, 2 * HD], BF, name=f"vA{c}") for c in range(NQC)
            ]
            aTs = [big.tile([P, 4, QC], BF, name=f"aT{c}") for c in range(NQC)]
            for c in range(NQC):
                nc.gpsimd.memset(vAs[c][:, :, :, HD : 2 * HD], 1.0)

            qT_of = {}

            def qkv_gen(j):
                """QKV projections for chunk j (single-term bf16); yields
                after each instruction so it can be pumped as PE filler
                during attention."""
                x = xs.pop(j)
                qT = qpool.tile([P, 4, QC], BF, tag="qtj")
                qT_of[j] = qT

                def qk(w, ob, dest, bias):
                    ps = psA.tile([P, QC], FP32, tag="psa", name="psqk")
                    for kp in range(8):
                        nc.tensor.matmul(
                            ps,
                            lhsT=w[:, kp, ob * P : (ob + 1) * P],
                            rhs=x[:, kp, :],
                            start=(kp == 0),
                            stop=(kp == 7),
                            skip_group_check=True,
                        )
                        yield
                    nc.vector.tensor_scalar_add(dest[:, ob, :], ps, bias[:, ob : ob + 1])
                    yield

                def v(sb):
                    psv = psA.tile([P, DHC], FP32, tag="psa", name="psv")
                    for kp in range(8):
                        nc.tensor.matmul(
                            psv,
                            lhsT=x[:, kp, sb * P : (sb + 1) * P],
                            rhs=w_t["wv"][:, kp, :],
                            start=(kp == 0),
                            stop=(kp == 7),
                            skip_group_check=True,
                        )
                        yield
                    nc.vector.tensor_add(
                        vAs[j][:, sb, :, 0:HD],
                        psv.rearrange("p (h d) -> p h d", d=HD),
                        bv_t.rearrange("p (h d) -> p h d", d=HD),
                    )
                    yield

                def q(ob):
                    yield from qk(w_t["wq"], ob, qT, bq_t)

                def k(ob):
                    yield from qk(w_t["wk"], ob, kTs[j], bk_t)

                if j == 0:
                    # chunk 0 is DMA-limited: consume tensors in arrival
                    # order (wq+x, wk, wv)
                    for ob in range(4):
                        yield from q(ob)
                    for ob in range(4):
                        yield from k(ob)
                    for sb in range(KPQ):
                        yield from v(sb)
                else:
                    # ob=0 of Q/K plus all of V first: that unblocks head 0 of
                    # the chunk's attention as early as possible.
                    yield from q(0)
                    yield from k(0)
                    for sb in range(KPQ):
                        yield from v(sb)
                    for ob in range(1, 4):
                        yield from q(ob)
                        yield from k(ob)

            def emit_out_store(j, sb, pp0, pp1, on_act, on_swdge=False):
                """Copy an nh pair of PSUM projection tiles into one row tile
                and store it as a single [128, 1024] DMA (fewer HWDGE preps)."""
                ot = opool.tile([P, 1024], BF, tag="ot", name="ot")
                if on_act:
                    nc.scalar.copy(out=ot[:, 0:512], in_=pp0)
                    nc.scalar.copy(out=ot[:, 512:1024], in_=pp1)
                else:
                    nc.vector.tensor_copy(out=ot[:, 0:512], in_=pp0)
                    nc.vector.tensor_copy(out=ot[:, 512:1024], in_=pp1)
                eng = nc.gpsimd if on_swdge else nc.sync
                eng.dma_start(
                    out[(j * KPQ + sb) * P : (j * KPQ + sb + 1) * P, :], ot
                )

            def outproj_gen(j):
                """Row-parallel output projection of chunk j's attn output."""
                for sb in range(KPQ):
                    pps = []
                    for nh in range(2):
                        pp = psA.tile([P, 512], FP32, tag="psa")
                        pps.append(pp)
                        for ib in range(4):
                            nc.tensor.matmul(
                                pp,
                                lhsT=aTs[j][:, ib, sb * P : (sb + 1) * P],
                                rhs=wp_t[:, ib, nh * 512 : (nh + 1) * 512],
                                start=(ib == 0),
                                stop=(ib == 3),
                                skip_group_check=True,
                            )
                            yield
                    emit_out_store(j, sb, pps[0], pps[1], on_act=False)
                    yield

            fillers = deque()

            def pump(n=1):
                while n > 0 and fillers:
                    try:
                        next(fillers[0])
                        n -= 1
                    except StopIteration:
                        fillers.popleft()

            def pump_gen(g):
                """Drain everything up to and including generator g."""
                while fillers and g in fillers:
                    try:
                        next(fillers[0])
                    except StopIteration:
                        fillers.popleft()

            # chunk 0's projections run inline (nothing to overlap yet)
            g0 = qkv_gen(0)
            fillers.append(g0)
            pump_gen(g0)

            for j in range(NQC):
                if j + 1 < NQC:
                    xs[j + 1] = xpool.tile(
                        [P, 8, QC], BF, tag="xt", name=f"xt{j + 1}"
                    )
                    nc.gpsimd.dma_start(xs[j + 1], xT[:, j + 1])
                    gnext = qkv_gen(j + 1)
                    fillers.appendleft(gnext)

                nk = KPQ * (j + 1)
                qT = qT_of[j]
                # Attention, software-pipelined by `lag` steps.  k-blocks are
                # processed in PAIRS: both scores land in one two-bank PSUM
                # tile and a single exp covers both (halving the per-
                # instruction ACT overhead — the exp stream is the local
                # bottleneck).  Each step emits its scores+exp+masks, then the
                # PV matmuls from `lag` steps earlier, so the PE always has
                # scores to run while an exp is in flight, including across
                # head boundaries.  pending holds (pv_fn, norm_fn) tuples;
                # norm_fn (the head's normalize) rides with its last PV.
                lag = 2 if j > 0 else 1
                pending = deque()

                def flush_one():
                    pv_fn, norm_fn = pending.popleft()
                    pv_fn()
                    if norm_fn is not None:
                        norm_fn()

                for h in range(HPC):
                    hb, ho = h // 2, (h % 2) * HD
                    po = psO.tile([P, QC], FP32, tag="po")
                    for pi in range(nk // 2):
                        ki0, ki1 = 2 * pi, 2 * pi + 1
                        t0 = ki0 - KPQ * j
                        # first valid q column per k-block (exact causal)
                        off0 = max(t0, 0) * P
                        off1 = max(t0 + 1, 0) * P
                        pair = psS.tile([P, 2, QC], FP32, tag="ps")
                        for i, (ki, off) in enumerate(((ki0, off0), (ki1, off1))):
                            kc, kb = divmod(ki, KPQ)
                            nc.tensor.matmul(
                                pair[:, i, off:],
                                lhsT=kTs[kc][ho : ho + HD, hb, kb * P : (kb + 1) * P],
                                rhs=qT[ho : ho + HD, hb, off:],
                                start=True,
                                stop=True,
                            )
                        ptp = ppool.tile([P, 2, QC], BF, tag="pt")
                        # one exp for both banks; the [off0:off1) slice of
                        # bank 1 is stale-score garbage that is exp'd but
                        # never read (PV/mask slice around it).
                        # scale 1/8 applies the 1/sqrt(HD) softmax scale.
                        nc.scalar.activation(
                            ptp[:, :, off0:], pair[:, :, off0:], Exp,
                            scale=2.0 ** -3,
                        )
                        if t0 >= 0:
                            # diag-block triangle mask on the (otherwise
                            # idle) Pool engine; DVE keeps the normalize path
                            nc.gpsimd.tensor_mul(
                                ptp[:, 0, off0 : off0 + P],
                                ptp[:, 0, off0 : off0 + P],
                                mk_t,
                            )
                            nc.gpsimd.tensor_mul(
                                ptp[:, 1, off1 : off1 + P],
                                ptp[:, 1, off1 : off1 + P],
                                mk_t,
                            )
                        if len(pending) >= lag:
                            flush_one()

                        def pv_fn(po=po, ki0=ki0, ki1=ki1, off0=off0,
                                  off1=off1, ptp=ptp, h=h, nk=nk):
                            for i, (ki, off) in enumerate(
                                ((ki0, off0), (ki1, off1))
                            ):
                                kc, kb = divmod(ki, KPQ)
                                nc.tensor.matmul(
                                    po[:, off:],
                                    lhsT=vAs[kc][:, kb, h, :],
                                    rhs=ptp[:, i, off:],
                                    start=(ki == 0),
                                    stop=(ki == nk - 1),
                                    skip_group_check=True,
                                )
                        norm_fn = None
                        if pi == nk // 2 - 1:
                            def norm_fn(j=j, h=h, hb=hb, ho=ho, po=po):
                                # denominator reciprocal as exp(-ln(d)) on
                                # ACT (Ln and Exp share one activation table
                                # so no table reloads; DVE's RECIPROCAL is
                                # 3.4us per head and blocks the po bank),
                                # then one DVE multiply straight into the
                                # bf16 attn-out tile.
                                rl = spool.tile([HD, QC], FP32, tag="rl",
                                                name="rl")
                                nc.scalar.activation(rl, po[HD : 2 * HD, :], Ln)
                                rr = spool.tile([HD, QC], FP32, tag="rr",
                                                name="rr")
                                nc.scalar.activation(rr, rl, Exp, scale=-1.0)
                                nc.vector.tensor_mul(
                                    aTs[j][ho : ho + HD, hb, :],
                                    po[0:HD, :],
                                    rr,
                                )
                        pending.append((pv_fn, norm_fn))
                        # two fillers per pair keeps the PE ahead of the
                        # exp stream (not in chunk 0, whose fillers wait on
                        # the xt(1) DMA)
                        pump(2 if j > 0 else 1)

                while pending:
                    flush_one()
                del qT_of[j]

                if j + 1 < NQC:
                    fillers.append(outproj_gen(j))
                    pump_gen(gnext)
                else:
                    # last chunk: drain leftovers, then run the final output
                    # projection with the ib=0..1 accumulations pre-started in
                    # borrowed PSUM banks so the PE overlaps the last head's
                    # normalize chain; ib=2..3 (heads 4-7) finishes after it.
                    # (a partial drain here deadlocks: held-back outproj
                    # generators' psA tiles circularly wait on the borrowed
                    # final-projection banks)
                    while fillers:
                        pump(1)
                    groups = [(sb, nh) for sb in range(KPQ) for nh in range(2)]
                    pps = []
                    fs = None
                    for g, (sb, nh) in enumerate(groups):
                        if g < 4:
                            if g % 2 == 0:
                                fs = psS.tile(
                                    [P, 2, 512], FP32, tag="ps", name=f"fppS{g // 2}"
                                )
                            pp = fs[:, g % 2, :]
                        elif g < 6:
                            pp = psA.tile([P, 512], FP32, tag="psa", name=f"fpp{g}")
                        else:
                            pp = psO.tile([P, QC], FP32, tag="po", name=f"fpp{g}")
                        pps.append(pp)
                        # ib 0..2 (heads 0-5) prestarts while the last heads'
                        # normalize chains complete; only ib=3 (heads 6/7)
                        # remains gated on the final normalize
                        for ib in range(3):
                            nc.tensor.matmul(
                                pp,
                                lhsT=aTs[j][:, ib, sb * P : (sb + 1) * P],
                                rhs=wp_t[:, ib, nh * 512 : (nh + 1) * 512],
                                start=(ib == 0),
                                stop=False,
                                skip_group_check=True,
                            )
                    for g, (sb, nh) in enumerate(groups):
                        nc.tensor.matmul(
                            pps[g],
                            lhsT=aTs[j][:, 3, sb * P : (sb + 1) * P],
                            rhs=wp_t[:, 3, nh * 512 : (nh + 1) * 512],
                            start=False,
                            stop=True,
                            skip_group_check=True,
                        )
                        if g % 2 == 1:
                            emit_out_store(
                                j, sb, pps[g - 1], pps[g],
                                on_act=(g % 4 == 3), on_swdge=(g % 4 == 3),
                            )

            while fillers:
                pump(1)


def build_program(seq=S, split=True):
    nc = bass.Bass("TRN2", target_bir_lowering=False, debug=False, num_devices=N_CORES)
    io = {
        "xt": nc.dram_tensor("xt", [P, (seq // QC) * 8 * QC], BF, kind="ExternalInput").ap(),
        "wq": nc.dram_tensor("wq", [P, 8 * DHC], BF, kind="ExternalInput").ap(),
        "wk": nc.dram_tensor("wk", [P, 8 * DHC], BF, kind="ExternalInput").ap(),
        "wv": nc.dram_tensor("wv", [P, 8 * DHC], BF, kind="ExternalInput").ap(),
        "wp": nc.dram_tensor("wp", [P, 4 * D], BF, kind="ExternalInput").ap(),
        "bq": nc.dram_tensor("bq", [P, 4], FP32, kind="ExternalInput").ap(),
        "bk": nc.dram_tensor("bk", [P, 4], FP32, kind="ExternalInput").ap(),
        "bv": nc.dram_tensor("bv", [P, DHC], FP32, kind="ExternalInput").ap(),
        "masks": nc.dram_tensor("masks", [P, P], BF, kind="ExternalInput").ap(),
        "out": nc.dram_tensor("out", [seq, D], BF, kind="ExternalOutput").ap(),
    }
    build_attention(nc, io, seq=seq)
    if split:
        split_excess_waits(nc)
    return nc


def make_masks():
    kk = np.arange(P)[:, None]
    qq = np.arange(P)[None, :]
    return np.ascontiguousarray((kk <= qq).astype(NPBF))


def blk_w(w):
    """(K, N) -> [128, (K//128)*N] with row ki holding all (ko, n) blocks."""
    k, n = w.shape
    return np.ascontiguousarray(
        w.reshape(k // P, P, n).transpose(1, 0, 2).reshape(P, (k // P) * n)
    )


def blk_x(xb):
    """x (S, D) -> chunk-major blocked x^T: [128, NQC*8*QC]."""
    seq = xb.shape[0]
    a = xb.T.reshape(8, P, seq)  # [ko, ki, s]
    b = a.transpose(1, 0, 2).reshape(P, 8, seq // QC, QC).transpose(0, 2, 1, 3)
    return np.ascontiguousarray(b.reshape(P, (seq // QC) * 8 * QC))


def shard_inputs(x, Wq, bq, Wk, bk, Wv, bv, Wp, bp, seq=S):
    masks = make_masks()
    in_maps = []
    xcache = {}
    for c in range(N_CORES):
        b, hg = c // 2, c % 2
        cols = slice(hg * DHC, (hg + 1) * DHC)
        bqc = np.ascontiguousarray(bq[cols].reshape(4, P).T)
        bkc = np.ascontiguousarray(bk[cols].reshape(4, P).T)
        bvc = np.ascontiguousarray(np.tile(bv[cols][None, :], (P, 1)))
        if b not in xcache:
            xcache[b] = blk_x(x[b]).astype(NPBF)
        in_maps.append(
            {
                "xt": xcache[b],
                "wq": blk_w(Wq[:, cols]).astype(NPBF),
                "wk": blk_w(Wk[:, cols]).astype(NPBF),
                "wv": blk_w(Wv[:, cols]).astype(NPBF),
                "wp": blk_w(Wp[cols, :]).astype(NPBF),
                "bq": bqc,
                "bk": bkc,
                "bv": bvc,
                "masks": masks,
            }
        )
    return in_maps


_NC_CACHE = {}


def _get_nc(seq=S):
    if seq not in _NC_CACHE:
        _NC_CACHE[seq] = build_program(seq)
    return _NC_CACHE[seq]


def kernel(x, Wq, bq, Wk, bk, Wv, bv, Wp, bp, **run_kwargs):
    from concourse.bass_utils import run_bass_kernel_spmd

    x = np.asarray(x, np.float32)
    Wq, Wk, Wv, Wp = (np.asarray(a, np.float32) for a in (Wq, Wk, Wv, Wp))
    bq, bk, bv, bp = (np.asarray(a, np.float32) for a in (bq, bk, bv, bp))

    nc = _get_nc()
    in_maps = shard_inputs(x, Wq, bq, Wk, bk, Wv, bv, Wp, bp)
    res = run_bass_kernel_spmd(nc, in_maps, core_ids=list(range(N_CORES)), **run_kwargs)
    parts = [np.asarray(res.results[c]["out"], np.float32) for c in range(N_CORES)]
    out = np.empty((B, S, D), np.float32)
    for b in range(B):
        out[b] = parts[2 * b] + parts[2 * b + 1] + bp
    kernel.last_results = res
    return out


# revision 17
# speedup vs baseline: 1.2884x; 1.0072x over previous
"""Causal self-attention on 8 TRN2 NeuronCores.

Sharding: core c handles batch b = c//2 and head-group hg = c%2 (8 of 16
heads).  Wq/Wk/Wv are split column-wise (per head-group), Wp row-wise; the
row-parallel partial outputs of the two cores sharing a batch are summed on
the host (cheap 8MB adds) together with the bp bias.

Per-core kernel (Tile framework, fp32 PSUM accum everywhere):
  phase A: Q^T, K^T (head-dim on partitions) and V (seq on partitions),
           plain bf16 matmuls (on HW, fp8 DoubleRow streams moving columns
           no faster than bf16, so a single bf16 term beats 3-term hi/lo
           fp8 by 1.5x).  V carries 64 ones-columns per head so the PV
           matmul emits the softmax denominator pre-broadcast across 64
           PSUM rows (matmul cost only depends on the moving free size, so
           the extra columns are free).
  phase B: per (head, q-chunk): scores^T = K Q^T (bf16) -> one exp per
           two-bank score pair (scale 1/8 applies the 1/sqrt(HD) score
           scale; flash-style, no max subtraction: scores ~ N(0,1)) ->
           causal mask (Pool engine; DVE stays clear) -> out^T accum =
           [V|1]^T p^T, PV software-pipelined 2 steps behind the scores ->
           normalize via ACT-engine reciprocal + DVE multiply straight
           into the bf16 attn-out tile.
  phase C: out_partial = attn_out^T.T @ Wp_slice (bf16, row-parallel),
           stored as bf16 and summed on the host.

Scheduling: projection matmuls (QKV of the next chunk, output projections
of finished chunks) are interleaved into the attention emission via filler
generators, keeping the PE fed.  Output projections are deferred to the
last (deepest) chunks; the final one pre-starts its ib=0..1 accumulations
in borrowed PSUM banks to overlap the last head's normalize.  Dummy warmup
matmuls during the startup DMAs hold the PE p-state at full clock.
"""

import sys

if "/opt/trn_rl_repo" not in sys.path:
    sys.path.insert(0, "/opt/trn_rl_repo")

from collections import deque
from contextlib import ExitStack

import ml_dtypes
import numpy as np

import concourse.bass as bass
import concourse.tile as tile
from concourse import mybir

P = 128
B, S, D, H = 4, 2048, 1024, 16
HD = 64          # head dim
HPC = 8          # heads per core
DHC = HPC * HD   # 512 inner dims per core
N_CORES = 8
QC = 512         # q-chunk width in phase B
FP32 = mybir.dt.float32
BF = mybir.dt.bfloat16
NPBF = ml_dtypes.bfloat16


def split_excess_waits(nc, max_waits=1):
    """walrus TPB_CTRL codegen in this container only accepts 1 sync-wait
    per instruction; hoist extras onto NoOps in front."""
    n = 0
    for fn in nc.m.functions:
        for bb in fn.blocks:
            il = bb.instructions
            i = 0
            while i < len(il):
                ins = il[i]
                si = getattr(ins, "sync_info", None)
                if si is not None and len(si.on_wait) > max_waits:
                    waits = list(si.on_wait)
                    keep = waits[-max_waits:]
                    extra = waits[:-max_waits]
                    for j in range(0, len(extra), max_waits):
                        nop = mybir.InstNoOp(
                            name=f"{ins.name}-wsplit{j}",
                            ins=[],
                            outs=[],
                            engine=ins.engine,
                            sync_info=mybir.SyncInfo(
                                on_wait=extra[j : j + max_waits], on_update=[]
                            ),
                        )
                        il.insert(i, nop)
                        i += 1
                        n += 1
                    si.on_wait = keep
                i += 1
    return n


def build_attention(nc, io, seq=S):
    Exp = mybir.ActivationFunctionType.Exp
    Ln = mybir.ActivationFunctionType.Ln
    NQC = seq // QC      # q chunks (also the pipeline step)
    KPQ = QC // P        # k-blocks per q chunk

    xT = io["xt"].rearrange("p (c k x) -> p c k x", k=8, x=QC)
    wqkv = {
        nm: io[nm].rearrange("p (k n) -> p k n", k=8)           # [128, 8, 512]
        for nm in ("wq", "wk", "wv")
    }
    wp = io["wp"].rearrange("p (k n) -> p k n", k=4)            # [128, 4, 1024]
    out = io["out"]                                             # [S, 1024]

    with ExitStack() as ctx:
        tc = ctx.enter_context(tile.TileContext(nc))
        const = ctx.enter_context(tc.tile_pool(name="const", bufs=1))
        big = ctx.enter_context(tc.tile_pool(name="big", bufs=1))

        with (
            tc.tile_pool(name="wqkv", bufs=1) as wpool,
            tc.tile_pool(name="xchunk", bufs=2) as xpool,
            tc.tile_pool(name="qtj", bufs=2) as qpool,
            tc.tile_pool(name="pt", bufs=6) as ppool,
            tc.tile_pool(name="small", bufs=3) as spool,
            tc.tile_pool(name="outp", bufs=6) as opool,
            tc.tile_pool(name="psA", bufs=2, space="PSUM") as psA,
            tc.tile_pool(name="psS", bufs=2, space="PSUM") as psS,
            tc.tile_pool(name="psO", bufs=2, space="PSUM") as psO,
        ):
            # ---- startup DMAs, most-urgent first (DMA engines serialize;
            # each sync-queue DMA also pays ~625ns of serialized HWDGE prep,
            # so small constants go on the gpsimd/SWDGE queue instead).
            # Order matches chunk-0 consumption: Q needs wq+x0, then wk, wv.
            # x chunks are split into kp-halves with one half on each DMA
            # queue (sync/HWDGE + gpsimd/SWDGE): both queues have ~10us of
            # cold-start latency, so splitting halves the arrival time of
            # the chunk-0 gate (wq + x0).
            def x_fetch(j):
                xa = xpool.tile([P, 4, QC], BF, tag="xta", name=f"xta{j}")
                nc.sync.dma_start(xa, xT[:, j, 0:4])
                xb = xpool.tile([P, 4, QC], BF, tag="xtb", name=f"xtb{j}")
                nc.gpsimd.dma_start(xb, xT[:, j, 4:8])
                return (xa, xb)

            w_t = {}
            w_t["wq"] = wpool.tile([P, 8, DHC], BF, name="wq")
            nc.sync.dma_start(w_t["wq"], wqkv["wq"])
            xs = {0: x_fetch(0)}
            w_t["wk"] = wpool.tile([P, 8, DHC], BF, name="wk")
            nc.sync.dma_start(w_t["wk"], wqkv["wk"])
            w_t["wv"] = wpool.tile([P, 8, DHC], BF, name="wv")
            nc.sync.dma_start(w_t["wv"], wqkv["wv"])
            wp_t = const.tile([P, 4, 1024], BF)
            nc.sync.dma_start(wp_t, wp)  # first needed by outproj(0), late
            # p-state warmup: the cost of a matmul ramps down only after ~3us
            # of continuous PE busy.  Run dummy matmuls on a zeroed tile while
            # the first wq/xt DMAs are in flight so the real projections start
            # at full clock with no ramp (and no >100ns issue gap to reset it).
            warm = const.tile([P, QC], BF)
            nc.gpsimd.memset(warm, 0.0)
            pswarm = psA.tile([P, QC], FP32, tag="psa", name="pswarm")
            for _ in range(12):
                nc.tensor.matmul(
                    pswarm,
                    lhsT=warm[:, 0:P],
                    rhs=warm,
                    start=True,
                    stop=True,
                    skip_group_check=True,
                )

            bq_t = const.tile([P, 4], FP32)
            nc.gpsimd.dma_start(bq_t, io["bq"])
            bk_t = const.tile([P, 4], FP32)
            nc.gpsimd.dma_start(bk_t, io["bk"])
            bv_t = const.tile([P, DHC], FP32)
            nc.gpsimd.dma_start(bv_t, io["bv"])
            mk_t = const.tile([P, P], BF)          # lower-triangular diag mask
            nc.gpsimd.dma_start(mk_t, io["masks"])

            # persistent per-chunk K^T, V and attn-out tiles.  V is augmented
            # with 64 ones-columns per head so the PV matmul emits the softmax
            # denominator already broadcast across 64 PSUM rows (matmul cost
            # only depends on the moving free size, so the extra columns are
            # free) — normalize needs no partition-broadcast DMA.
            kTs = [big.tile([P, 4, QC], BF, name=f"kT{c}") for c in range(NQC)]
            vAs = [
                big.tile([P, KPQ, HPC, 2 * HD], BF, name=f"vA{c}") for c in range(NQC)
            ]
            aTs = [big.tile([P, 4, QC], BF, name=f"aT{c}") for c in range(NQC)]
            for c in range(NQC):
                nc.gpsimd.memset(vAs[c][:, :, :, HD : 2 * HD], 1.0)

            qT_of = {}

            def qkv_gen(j):
                """QKV projections for chunk j (single-term bf16); yields
                after each instruction so it can be pumped as PE filler
                during attention."""
                xab = xs.pop(j)
                qT = qpool.tile([P, 4, QC], BF, tag="qtj")
                qT_of[j] = qT

                def qk(w, ob, dest, bias):
                    ps = psA.tile([P, QC], FP32, tag="psa", name="psqk")
                    for kp in range(8):
                        nc.tensor.matmul(
                            ps,
                            lhsT=w[:, kp, ob * P : (ob + 1) * P],
                            rhs=xab[kp // 4][:, kp % 4, :],
                            start=(kp == 0),
                            stop=(kp == 7),
                            skip_group_check=True,
                        )
                        yield
                    nc.vector.tensor_scalar_add(dest[:, ob, :], ps, bias[:, ob : ob + 1])
                    yield

                def v(sb):
                    psv = psA.tile([P, DHC], FP32, tag="psa", name="psv")
                    for kp in range(8):
                        nc.tensor.matmul(
                            psv,
                            lhsT=xab[kp // 4][:, kp % 4, sb * P : (sb + 1) * P],
                            rhs=w_t["wv"][:, kp, :],
                            start=(kp == 0),
                            stop=(kp == 7),
                            skip_group_check=True,
                        )
                        yield
                    nc.vector.tensor_add(
                        vAs[j][:, sb, :, 0:HD],
                        psv.rearrange("p (h d) -> p h d", d=HD),
                        bv_t.rearrange("p (h d) -> p h d", d=HD),
                    )
                    yield

                def q(ob):
                    yield from qk(w_t["wq"], ob, qT, bq_t)

                def k(ob):
                    yield from qk(w_t["wk"], ob, kTs[j], bk_t)

                if j == 0:
                    # chunk 0 is DMA-limited: consume tensors in arrival
                    # order (wq+x, wk, wv)
                    for ob in range(4):
                        yield from q(ob)
                    for ob in range(4):
                        yield from k(ob)
                    for sb in range(KPQ):
                        yield from v(sb)
                else:
                    # ob=0 of Q/K plus all of V first: that unblocks head 0 of
                    # the chunk's attention as early as possible.
                    yield from q(0)
                    yield from k(0)
                    for sb in range(KPQ):
                        yield from v(sb)
                    for ob in range(1, 4):
                        yield from q(ob)
                        yield from k(ob)

            def emit_out_store(j, sb, pp0, pp1, on_act, on_swdge=False,
                               split=False):
                """Copy an nh pair of PSUM projection tiles into one row tile
                and store it as a single [128, 1024] DMA (fewer HWDGE preps).
                split=True puts the two copies on different engines
                (DVE + ACT; Pool can't read PSUM) to halve the serial copy
                chain in the tail."""
                ot = opool.tile([P, 1024], BF, tag="ot", name="ot")
                if split:
                    nc.vector.tensor_copy(out=ot[:, 0:512], in_=pp0)
                    nc.scalar.copy(out=ot[:, 512:1024], in_=pp1)
                elif on_act:
                    nc.scalar.copy(out=ot[:, 0:512], in_=pp0)
                    nc.scalar.copy(out=ot[:, 512:1024], in_=pp1)
                else:
                    nc.vector.tensor_copy(out=ot[:, 0:512], in_=pp0)
                    nc.vector.tensor_copy(out=ot[:, 512:1024], in_=pp1)
                eng = nc.gpsimd if on_swdge else nc.sync
                eng.dma_start(
                    out[(j * KPQ + sb) * P : (j * KPQ + sb + 1) * P, :], ot
                )

            def outproj_gen(j):
                """Row-parallel output projection of chunk j's attn output."""
                for sb in range(KPQ):
                    pps = []
                    for nh in range(2):
                        pp = psA.tile([P, 512], FP32, tag="psa")
                        pps.append(pp)
                        for ib in range(4):
                            nc.tensor.matmul(
                                pp,
                                lhsT=aTs[j][:, ib, sb * P : (sb + 1) * P],
                                rhs=wp_t[:, ib, nh * 512 : (nh + 1) * 512],
                                start=(ib == 0),
                                stop=(ib == 3),
                                skip_group_check=True,
                            )
                            yield
                    emit_out_store(j, sb, pps[0], pps[1], on_act=False)
                    yield

            fillers = deque()

            def pump(n=1):
                while n > 0 and fillers:
                    try:
                        next(fillers[0])
                        n -= 1
                    except StopIteration:
                        fillers.popleft()

            def pump_gen(g):
                """Drain everything up to and including generator g."""
                while fillers and g in fillers:
                    try:
                        next(fillers[0])
                    except StopIteration:
                        fillers.popleft()

            # chunk 0's projections run inline (nothing to overlap yet)
            g0 = qkv_gen(0)
            fillers.append(g0)
            pump_gen(g0)

            for j in range(NQC):
                if j + 1 < NQC:
                    xs[j + 1] = x_fetch(j + 1)
                    gnext = qkv_gen(j + 1)
                    fillers.appendleft(gnext)

                nk = KPQ * (j + 1)
                qT = qT_of[j]
                # Attention, software-pipelined by `lag` steps.  k-blocks are
                # processed in PAIRS: both scores land in one two-bank PSUM
                # tile and a single exp covers both (halving the per-
                # instruction ACT overhead — the exp stream is the local
                # bottleneck).  Each step emits its scores+exp+masks, then the
                # PV matmuls from `lag` steps earlier, so the PE always has
                # scores to run while an exp is in flight, including across
                # head boundaries.  pending holds (pv_fn, norm_fn) tuples;
                # norm_fn (the head's normalize) rides with its last PV.
                lag = 2 if j > 0 else 1
                pending = deque()

                def flush_one():
                    pv_fn, norm_fn = pending.popleft()
                    pv_fn()
                    if norm_fn is not None:
                        norm_fn()

                for h in range(HPC):
                    hb, ho = h // 2, (h % 2) * HD
                    po = psO.tile([P, QC], FP32, tag="po")
                    for pi in range(nk // 2):
                        ki0, ki1 = 2 * pi, 2 * pi + 1
                        t0 = ki0 - KPQ * j
                        # first valid q column per k-block (exact causal)
                        off0 = max(t0, 0) * P
                        off1 = max(t0 + 1, 0) * P
                        pair = psS.tile([P, 2, QC], FP32, tag="ps")
                        for i, (ki, off) in enumerate(((ki0, off0), (ki1, off1))):
                            kc, kb = divmod(ki, KPQ)
                            nc.tensor.matmul(
                                pair[:, i, off:],
                                lhsT=kTs[kc][ho : ho + HD, hb, kb * P : (kb + 1) * P],
                                rhs=qT[ho : ho + HD, hb, off:],
                                start=True,
                                stop=True,
                            )
                        ptp = ppool.tile([P, 2, QC], BF, tag="pt")
                        # one exp for both banks; the [off0:off1) slice of
                        # bank 1 is stale-score garbage that is exp'd but
                        # never read (PV/mask slice around it).
                        # scale 1/8 applies the 1/sqrt(HD) softmax scale.
                        nc.scalar.activation(
                            ptp[:, :, off0:], pair[:, :, off0:], Exp,
                            scale=2.0 ** -3,
                        )
                        if t0 >= 0:
                            # diag-block triangle mask on the (otherwise
                            # idle) Pool engine; DVE keeps the normalize path
                            nc.gpsimd.tensor_mul(
                                ptp[:, 0, off0 : off0 + P],
                                ptp[:, 0, off0 : off0 + P],
                                mk_t,
                            )
                            nc.gpsimd.tensor_mul(
                                ptp[:, 1, off1 : off1 + P],
                                ptp[:, 1, off1 : off1 + P],
                                mk_t,
                            )
                        if len(pending) >= lag:
                            flush_one()

                        def pv_fn(po=po, ki0=ki0, ki1=ki1, off0=off0,
                                  off1=off1, ptp=ptp, h=h, nk=nk):
                            for i, (ki, off) in enumerate(
                                ((ki0, off0), (ki1, off1))
                            ):
                                kc, kb = divmod(ki, KPQ)
                                nc.tensor.matmul(
                                    po[:, off:],
                                    lhsT=vAs[kc][:, kb, h, :],
                                    rhs=ptp[:, i, off:],
                                    start=(ki == 0),
                                    stop=(ki == nk - 1),
                                    skip_group_check=True,
                                )
                        norm_fn = None
                        if pi == nk // 2 - 1:
                            def norm_fn(j=j, h=h, hb=hb, ho=ho, po=po):
                                # denominator reciprocal as exp(-ln(d)) on
                                # ACT (Ln and Exp share one activation table
                                # so no table reloads; DVE's RECIPROCAL is
                                # 3.4us per head and blocks the po bank),
                                # then one DVE multiply straight into the
                                # bf16 attn-out tile.
                                rl = spool.tile([HD, QC], FP32, tag="rl",
                                                name="rl")
                                nc.scalar.activation(rl, po[HD : 2 * HD, :], Ln)
                                rr = spool.tile([HD, QC], FP32, tag="rr",
                                                name="rr")
                                nc.scalar.activation(rr, rl, Exp, scale=-1.0)
                                nc.vector.tensor_mul(
                                    aTs[j][ho : ho + HD, hb, :],
                                    po[0:HD, :],
                                    rr,
                                )
                        pending.append((pv_fn, norm_fn))
                        # two fillers per pair keeps the PE ahead of the
                        # exp stream (not in chunk 0, whose fillers wait on
                        # the xt(1) DMA)
                        pump(2 if j > 0 else 1)

                while pending:
                    flush_one()
                del qT_of[j]

                if j + 1 < NQC:
                    # chunk 1's output projection is deferred to chunk 3:
                    # chunk 2 is PE-bound (its own attention + chunk-3 QKV
                    # fillers) while chunk 3 is exp/ACT-bound with PE slack.
                    if j == 0:
                        fillers.append(outproj_gen(0))
                    elif j == 2:
                        fillers.append(outproj_gen(1))
                        fillers.append(outproj_gen(2))
                    pump_gen(gnext)
                else:
                    # last chunk: drain leftovers, then run the final output
                    # projection with the ib=0..1 accumulations pre-started in
                    # borrowed PSUM banks so the PE overlaps the last head's
                    # normalize chain; ib=2..3 (heads 4-7) finishes after it.
                    # (a partial drain here deadlocks: held-back outproj
                    # generators' psA tiles circularly wait on the borrowed
                    # final-projection banks)
                    while fillers:
                        pump(1)
                    groups = [(sb, nh) for sb in range(KPQ) for nh in range(2)]
                    pps = []
                    fs = None
                    for g, (sb, nh) in enumerate(groups):
                        if g < 4:
                            if g % 2 == 0:
                                fs = psS.tile(
                                    [P, 2, 512], FP32, tag="ps", name=f"fppS{g // 2}"
                                )
                            pp = fs[:, g % 2, :]
                        elif g < 6:
                            pp = psA.tile([P, 512], FP32, tag="psa", name=f"fpp{g}")
                        else:
                            pp = psO.tile([P, QC], FP32, tag="po", name=f"fpp{g}")
                        pps.append(pp)
                        # ib 0..2 (heads 0-5) prestarts while the last heads'
                        # normalize chains complete; only ib=3 (heads 6/7)
                        # remains gated on the final normalize
                        for ib in range(3):
                            nc.tensor.matmul(
                                pp,
                                lhsT=aTs[j][:, ib, sb * P : (sb + 1) * P],
                                rhs=wp_t[:, ib, nh * 512 : (nh + 1) * 512],
                                start=(ib == 0),
                                stop=False,
                                skip_group_check=True,
                            )
                    for g, (sb, nh) in enumerate(groups):
                        nc.tensor.matmul(
                            pps[g],
                            lhsT=aTs[j][:, 3, sb * P : (sb + 1) * P],
                            rhs=wp_t[:, 3, nh * 512 : (nh + 1) * 512],
                            start=False,
                            stop=True,
                            skip_group_check=True,
                        )
                        if g % 2 == 1:
                            emit_out_store(
                                j, sb, pps[g - 1], pps[g],
                                on_act=(g % 4 == 3), on_swdge=(g % 4 == 3),
                                split=True,
                            )

            while fillers:
                pump(1)


def build_program(seq=S, split=True):
    nc = bass.Bass("TRN2", target_bir_lowering=False, debug=False, num_devices=N_CORES)
    io = {
        "xt": nc.dram_tensor("xt", [P, (seq // QC) * 8 * QC], BF, kind="ExternalInput").ap(),
        "wq": nc.dram_tensor("wq", [P, 8 * DHC], BF, kind="ExternalInput").ap(),
        "wk": nc.dram_tensor("wk", [P, 8 * DHC], BF, kind="ExternalInput").ap(),
        "wv": nc.dram_tensor("wv", [P, 8 * DHC], BF, kind="ExternalInput").ap(),
        "wp": nc.dram_tensor("wp", [P, 4 * D], BF, kind="ExternalInput").ap(),
        "bq": nc.dram_tensor("bq", [P, 4], FP32, kind="ExternalInput").ap(),
        "bk": nc.dram_tensor("bk", [P, 4], FP32, kind="ExternalInput").ap(),
        "bv": nc.dram_tensor("bv", [P, DHC], FP32, kind="ExternalInput").ap(),
        "masks": nc.dram_tensor("masks", [P, P], BF, kind="ExternalInput").ap(),
        "out": nc.dram_tensor("out", [seq, D], BF, kind="ExternalOutput").ap(),
    }
    build_attention(nc, io, seq=seq)
    if split:
        split_excess_waits(nc)
    return nc


def make_masks():
    kk = np.arange(P)[:, None]
    qq = np.arange(P)[None, :]
    return np.ascontiguousarray((kk <= qq).astype(NPBF))


def blk_w(w):
    """(K, N) -> [128, (K//128)*N] with row ki holding all (ko, n) blocks."""
    k, n = w.shape
    return np.ascontiguousarray(
        w.reshape(k // P, P, n).transpose(1, 0, 2).reshape(P, (k // P) * n)
    )


def blk_x(xb):
    """x (S, D) -> chunk-major blocked x^T: [128, NQC*8*QC]."""
    seq = xb.shape[0]
    a = xb.T.reshape(8, P, seq)  # [ko, ki, s]
    b = a.transpose(1, 0, 2).reshape(P, 8, seq // QC, QC).transpose(0, 2, 1, 3)
    return np.ascontiguousarray(b.reshape(P, (seq // QC) * 8 * QC))


def shard_inputs(x, Wq, bq, Wk, bk, Wv, bv, Wp, bp, seq=S):
    masks = make_masks()
    in_maps = []
    xcache = {}
    for c in range(N_CORES):
        b, hg = c // 2, c % 2
        cols = slice(hg * DHC, (hg + 1) * DHC)
        bqc = np.ascontiguousarray(bq[cols].reshape(4, P).T)
        bkc = np.ascontiguousarray(bk[cols].reshape(4, P).T)
        bvc = np.ascontiguousarray(np.tile(bv[cols][None, :], (P, 1)))
        if b not in xcache:
            xcache[b] = blk_x(x[b]).astype(NPBF)
        in_maps.append(
            {
                "xt": xcache[b],
                "wq": blk_w(Wq[:, cols]).astype(NPBF),
                "wk": blk_w(Wk[:, cols]).astype(NPBF),
                "wv": blk_w(Wv[:, cols]).astype(NPBF),
                "wp": blk_w(Wp[cols, :]).astype(NPBF),
                "bq": bqc,
                "bk": bkc,
                "bv": bvc,
                "masks": masks,
            }
        )
    return in_maps


_NC_CACHE = {}


def _get_nc(seq=S):
    if seq not in _NC_CACHE:
        _NC_CACHE[seq] = build_program(seq)
    return _NC_CACHE[seq]


def kernel(x, Wq, bq, Wk, bk, Wv, bv, Wp, bp, **run_kwargs):
    from concourse.bass_utils import run_bass_kernel_spmd

    x = np.asarray(x, np.float32)
    Wq, Wk, Wv, Wp = (np.asarray(a, np.float32) for a in (Wq, Wk, Wv, Wp))
    bq, bk, bv, bp = (np.asarray(a, np.float32) for a in (bq, bk, bv, bp))

    nc = _get_nc()
    in_maps = shard_inputs(x, Wq, bq, Wk, bk, Wv, bv, Wp, bp)
    res = run_bass_kernel_spmd(nc, in_maps, core_ids=list(range(N_CORES)), **run_kwargs)
    parts = [np.asarray(res.results[c]["out"], np.float32) for c in range(N_CORES)]
    out = np.empty((B, S, D), np.float32)
    for b in range(B):
        out[b] = parts[2 * b] + parts[2 * b + 1] + bp
    kernel.last_results = res
    return out


# revision 23
# speedup vs baseline: 1.3062x; 1.0138x over previous
"""Causal self-attention on 8 TRN2 NeuronCores.

Sharding: core c handles batch b = c//2 and head-group hg = c%2 (8 of 16
heads).  Wq/Wk/Wv are split column-wise (per head-group), Wp row-wise; the
row-parallel partial outputs of the two cores sharing a batch are summed on
the host (cheap 8MB adds) together with the bp bias.

Per-core kernel (Tile framework, fp32 PSUM accum everywhere):
  phase A: Q^T, K^T (head-dim on partitions) and V (seq on partitions),
           plain bf16 matmuls (on HW, fp8 DoubleRow streams moving columns
           no faster than bf16, so a single bf16 term beats 3-term hi/lo
           fp8 by 1.5x).  V carries 64 ones-columns per head so the PV
           matmul emits the softmax denominator pre-broadcast across 64
           PSUM rows (matmul cost only depends on the moving free size, so
           the extra columns are free).
  phase B: per (head, q-chunk): scores^T = K Q^T (bf16) -> one exp per
           two-bank score pair (scale 1/8 applies the 1/sqrt(HD) score
           scale; flash-style, no max subtraction: scores ~ N(0,1)) ->
           causal mask (Pool engine; DVE stays clear) -> out^T accum =
           [V|1]^T p^T, PV software-pipelined 2 steps behind the scores ->
           normalize via ACT-engine reciprocal + DVE multiply straight
           into the bf16 attn-out tile.
  phase C: out_partial = attn_out^T.T @ Wp_slice (bf16, row-parallel),
           stored as bf16 and summed on the host.

Scheduling: projection matmuls (QKV of the next chunk, output projections
of finished chunks) are interleaved into the attention emission via filler
generators, keeping the PE fed.  Output projections are deferred to the
last (deepest) chunks; the final one pre-starts its ib=0..1 accumulations
in borrowed PSUM banks to overlap the last head's normalize.  Dummy warmup
matmuls during the startup DMAs hold the PE p-state at full clock.
"""

import sys

if "/opt/trn_rl_repo" not in sys.path:
    sys.path.insert(0, "/opt/trn_rl_repo")

from collections import deque
from contextlib import ExitStack

import ml_dtypes
import numpy as np

import concourse.bass as bass
import concourse.tile as tile
from concourse import mybir

P = 128
B, S, D, H = 4, 2048, 1024, 16
HD = 64          # head dim
HPC = 8          # heads per core
DHC = HPC * HD   # 512 inner dims per core
N_CORES = 8
QC = 512         # q-chunk width in phase B
FP32 = mybir.dt.float32
BF = mybir.dt.bfloat16
NPBF = ml_dtypes.bfloat16


def split_excess_waits(nc, max_waits=1):
    """walrus TPB_CTRL codegen in this container only accepts 1 sync-wait
    per instruction; hoist extras onto NoOps in front."""
    n = 0
    for fn in nc.m.functions:
        for bb in fn.blocks:
            il = bb.instructions
            i = 0
            while i < len(il):
                ins = il[i]
                si = getattr(ins, "sync_info", None)
                if si is not None and len(si.on_wait) > max_waits:
                    waits = list(si.on_wait)
                    keep = waits[-max_waits:]
                    extra = waits[:-max_waits]
                    for j in range(0, len(extra), max_waits):
                        nop = mybir.InstNoOp(
                            name=f"{ins.name}-wsplit{j}",
                            ins=[],
                            outs=[],
                            engine=ins.engine,
                            sync_info=mybir.SyncInfo(
                                on_wait=extra[j : j + max_waits], on_update=[]
                            ),
                        )
                        il.insert(i, nop)
                        i += 1
                        n += 1
                    si.on_wait = keep
                i += 1
    return n


def build_attention(nc, io, seq=S):
    Exp = mybir.ActivationFunctionType.Exp
    Ln = mybir.ActivationFunctionType.Ln
    NQC = seq // QC      # q chunks (also the pipeline step)
    KPQ = QC // P        # k-blocks per q chunk

    xT = io["xt"].rearrange("p (c k x) -> p c k x", k=8, x=QC)
    wqkv = {
        nm: io[nm].rearrange("p (k n) -> p k n", k=8)           # [128, 8, 512]
        for nm in ("wq", "wk", "wv")
    }
    wp = io["wp"].rearrange("p (k n) -> p k n", k=4)            # [128, 4, 1024]
    out = io["out"]                                             # [S, 1024]

    with ExitStack() as ctx:
        tc = ctx.enter_context(tile.TileContext(nc))
        const = ctx.enter_context(tc.tile_pool(name="const", bufs=1))
        big = ctx.enter_context(tc.tile_pool(name="big", bufs=1))

        with (
            tc.tile_pool(name="wqkv", bufs=1) as wpool,
            tc.tile_pool(name="xchunk", bufs=2) as xpool,
            tc.tile_pool(name="qtj", bufs=2) as qpool,
            tc.tile_pool(name="pt", bufs=6) as ppool,
            tc.tile_pool(name="small", bufs=3) as spool,
            tc.tile_pool(name="outp", bufs=6) as opool,
            tc.tile_pool(name="psA", bufs=2, space="PSUM") as psA,
            tc.tile_pool(name="psS", bufs=2, space="PSUM") as psS,
            tc.tile_pool(name="psO", bufs=2, space="PSUM") as psO,
        ):
            # ---- startup DMAs, most-urgent first (DMA engines serialize;
            # each sync-queue DMA also pays ~625ns of serialized HWDGE prep,
            # so small constants go on the gpsimd/SWDGE queue instead).
            # Order matches chunk-0 consumption: Q needs wq+x0, then wk, wv.
            # x chunks are split into kp-halves with one half on each DMA
            # queue (sync/HWDGE + gpsimd/SWDGE): both queues have ~10us of
            # cold-start latency, so splitting halves the arrival time of
            # the chunk-0 gate (wq + x0).
            def x_fetch(j):
                xa = xpool.tile([P, 4, QC], BF, tag="xta", name=f"xta{j}")
                nc.sync.dma_start(xa, xT[:, j, 0:4])
                xb = xpool.tile([P, 4, QC], BF, tag="xtb", name=f"xtb{j}")
                nc.gpsimd.dma_start(xb, xT[:, j, 4:8])
                return (xa, xb)

            # wq/wk are also kp-halved so chunk-0 Q matmuls (needing only
            # wq_a + x0a) start ~3us earlier on the serialized sync queue
            w_t = {}
            w_t["wq"] = wpool.tile([P, 8, DHC], BF, name="wq")
            nc.sync.dma_start(w_t["wq"][:, 0:4], wqkv["wq"][:, 0:4])
            xs = {0: x_fetch(0)}
            nc.sync.dma_start(w_t["wq"][:, 4:8], wqkv["wq"][:, 4:8])
            w_t["wk"] = wpool.tile([P, 8, DHC], BF, name="wk")
            nc.sync.dma_start(w_t["wk"][:, 0:4], wqkv["wk"][:, 0:4])
            nc.sync.dma_start(w_t["wk"][:, 4:8], wqkv["wk"][:, 4:8])
            w_t["wv"] = wpool.tile([P, 8, DHC], BF, name="wv")
            nc.sync.dma_start(w_t["wv"], wqkv["wv"])
            wp_t = const.tile([P, 4, 1024], BF)
            nc.sync.dma_start(wp_t, wp)  # first needed by outproj(0), late
            # p-state warmup: the cost of a matmul ramps down only after ~3us
            # of continuous PE busy.  Run dummy matmuls on a zeroed tile while
            # the first wq/xt DMAs are in flight so the real projections start
            # at full clock with no ramp (and no >100ns issue gap to reset it).
            warm = const.tile([P, QC], BF)
            nc.gpsimd.memset(warm, 0.0)
            pswarm = psA.tile([P, QC], FP32, tag="psa", name="pswarm")
            for _ in range(12):
                nc.tensor.matmul(
                    pswarm,
                    lhsT=warm[:, 0:P],
                    rhs=warm,
                    start=True,
                    stop=True,
                    skip_group_check=True,
                )

            bq_t = const.tile([P, 4], FP32)
            nc.gpsimd.dma_start(bq_t, io["bq"])
            bk_t = const.tile([P, 4], FP32)
            nc.gpsimd.dma_start(bk_t, io["bk"])
            bv_t = const.tile([P, DHC], FP32)
            nc.gpsimd.dma_start(bv_t, io["bv"])
            mk_t = const.tile([P, P], BF)          # lower-triangular diag mask
            nc.gpsimd.dma_start(mk_t, io["masks"])

            # persistent per-chunk K^T, V and attn-out tiles.  V is augmented
            # with 64 ones-columns per head so the PV matmul emits the softmax
            # denominator already broadcast across 64 PSUM rows (matmul cost
            # only depends on the moving free size, so the extra columns are
            # free) — normalize needs no partition-broadcast DMA.
            kTs = [big.tile([P, 4, QC], BF, name=f"kT{c}") for c in range(NQC)]
            vAs = [
                big.tile([P, KPQ, HPC, 2 * HD], BF, name=f"vA{c}") for c in range(NQC)
            ]
            aTs = [big.tile([P, 4, QC], BF, name=f"aT{c}") for c in range(NQC)]
            for c in range(NQC):
                nc.gpsimd.memset(vAs[c][:, :, :, HD : 2 * HD], 1.0)

            qT_of = {}

            def qkv_gen(j):
                """QKV projections for chunk j (single-term bf16); yields
                after each instruction so it can be pumped as PE filler
                during attention."""
                xab = xs.pop(j)
                qT = qpool.tile([P, 4, QC], BF, tag="qtj")
                qT_of[j] = qT
                # chunk 0's xa half rides the (earlier-starting) sync queue,
                # so consume kp 0-3 first; later chunks' xa halves queue
                # behind the weights on sync while xb lands early on SWDGE,
                # so consume kp 4-7 first (accumulation order is free).
                kps = range(8) if j == 0 else [4, 5, 6, 7, 0, 1, 2, 3]

                def qk(w, ob, dest, bias):
                    ps = psA.tile([P, QC], FP32, tag="psa", name="psqk")
                    for i, kp in enumerate(kps):
                        nc.tensor.matmul(
                            ps,
                            lhsT=w[:, kp, ob * P : (ob + 1) * P],
                            rhs=xab[kp // 4][:, kp % 4, :],
                            start=(i == 0),
                            stop=(i == 7),
                            skip_group_check=True,
                        )
                        yield
                    nc.vector.tensor_scalar_add(dest[:, ob, :], ps, bias[:, ob : ob + 1])
                    yield

                def v(sb):
                    psv = psA.tile([P, DHC], FP32, tag="psa", name="psv")
                    for i, kp in enumerate(kps):
                        nc.tensor.matmul(
                            psv,
                            lhsT=xab[kp // 4][:, kp % 4, sb * P : (sb + 1) * P],
                            rhs=w_t["wv"][:, kp, :],
                            start=(i == 0),
                            stop=(i == 7),
                            skip_group_check=True,
                        )
                        yield
                    nc.vector.tensor_add(
                        vAs[j][:, sb, :, 0:HD],
                        psv.rearrange("p (h d) -> p h d", d=HD),
                        bv_t.rearrange("p (h d) -> p h d", d=HD),
                    )
                    yield

                def q(ob):
                    yield from qk(w_t["wq"], ob, qT, bq_t)

                def k(ob):
                    yield from qk(w_t["wk"], ob, kTs[j], bk_t)

                if j == 0:
                    # chunk 0 is DMA-limited: consume tensors in arrival
                    # order (wq+x, wk, wv)
                    for ob in range(4):
                        yield from q(ob)
                    for ob in range(4):
                        yield from k(ob)
                    for sb in range(KPQ):
                        yield from v(sb)
                else:
                    # ob=0 of Q/K plus all of V first: that unblocks head 0 of
                    # the chunk's attention as early as possible.
                    yield from q(0)
                    yield from k(0)
                    for sb in range(KPQ):
                        yield from v(sb)
                    for ob in range(1, 4):
                        yield from q(ob)
                        yield from k(ob)

            def emit_out_store(j, sb, pp0, pp1, on_act, on_swdge=False,
                               split=False):
                """Copy an nh pair of PSUM projection tiles into one row tile
                and store it as a single [128, 1024] DMA (fewer HWDGE preps).
                split=True puts the two copies on different engines
                (DVE + ACT; Pool can't read PSUM) to halve the serial copy
                chain in the tail."""
                ot = opool.tile([P, 1024], BF, tag="ot", name="ot")
                if split:
                    nc.vector.tensor_copy(out=ot[:, 0:512], in_=pp0)
                    nc.scalar.copy(out=ot[:, 512:1024], in_=pp1)
                elif on_act:
                    nc.scalar.copy(out=ot[:, 0:512], in_=pp0)
                    nc.scalar.copy(out=ot[:, 512:1024], in_=pp1)
                else:
                    nc.vector.tensor_copy(out=ot[:, 0:512], in_=pp0)
                    nc.vector.tensor_copy(out=ot[:, 512:1024], in_=pp1)
                eng = nc.gpsimd if on_swdge else nc.sync
                eng.dma_start(
                    out[(j * KPQ + sb) * P : (j * KPQ + sb + 1) * P, :], ot
                )

            def outproj_gen(j):
                """Row-parallel output projection of chunk j's attn output."""
                for sb in range(KPQ):
                    pps = []
                    for nh in range(2):
                        pp = psA.tile([P, 512], FP32, tag="psa")
                        pps.append(pp)
                        for ib in range(4):
                            nc.tensor.matmul(
                                pp,
                                lhsT=aTs[j][:, ib, sb * P : (sb + 1) * P],
                                rhs=wp_t[:, ib, nh * 512 : (nh + 1) * 512],
                                start=(ib == 0),
                                stop=(ib == 3),
                                skip_group_check=True,
                            )
                            yield
                    emit_out_store(j, sb, pps[0], pps[1], on_act=False)
                    yield

            fillers = deque()

            def pump(n=1):
                while n > 0 and fillers:
                    try:
                        next(fillers[0])
                        n -= 1
                    except StopIteration:
                        fillers.popleft()

            def pump_gen(g):
                """Drain everything up to and including generator g."""
                while fillers and g in fillers:
                    try:
                        next(fillers[0])
                    except StopIteration:
                        fillers.popleft()

            # chunk 0's projections run inline (nothing to overlap yet)
            g0 = qkv_gen(0)
            fillers.append(g0)
            pump_gen(g0)

            for j in range(NQC):
                if j + 1 < NQC:
                    xs[j + 1] = x_fetch(j + 1)
                    gnext = qkv_gen(j + 1)
                    fillers.appendleft(gnext)

                nk = KPQ * (j + 1)
                qT = qT_of[j]
                # Attention, software-pipelined by `lag` steps.  k-blocks are
                # processed in PAIRS: both scores land in one two-bank PSUM
                # tile and a single exp covers both (halving the per-
                # instruction ACT overhead — the exp stream is the local
                # bottleneck).  Each step emits its scores+exp+masks, then the
                # PV matmuls from `lag` steps earlier, so the PE always has
                # scores to run while an exp is in flight, including across
                # head boundaries.  pending holds (pv_fn, norm_fn) tuples;
                # norm_fn (the head's normalize) rides with its last PV.
                lag = 2 if j > 0 else 1
                pending = deque()

                def flush_one():
                    pv_fn, norm_fn = pending.popleft()
                    pv_fn()
                    if norm_fn is not None:
                        norm_fn()

                for h in range(HPC):
                    hb, ho = h // 2, (h % 2) * HD
                    po = psO.tile([P, QC], FP32, tag="po")
                    for pi in range(nk // 2):
                        ki0, ki1 = 2 * pi, 2 * pi + 1
                        t0 = ki0 - KPQ * j
                        # first valid q column per k-block (exact causal)
                        off0 = max(t0, 0) * P
                        off1 = max(t0 + 1, 0) * P
                        pair = psS.tile([P, 2, QC], FP32, tag="ps")
                        for i, (ki, off) in enumerate(((ki0, off0), (ki1, off1))):
                            kc, kb = divmod(ki, KPQ)
                            nc.tensor.matmul(
                                pair[:, i, off:],
                                lhsT=kTs[kc][ho : ho + HD, hb, kb * P : (kb + 1) * P],
                                rhs=qT[ho : ho + HD, hb, off:],
                                start=True,
                                stop=True,
                            )
                        ptp = ppool.tile([P, 2, QC], BF, tag="pt")
                        # one exp for both banks; the [off0:off1) slice of
                        # bank 1 is stale-score garbage that is exp'd but
                        # never read (PV/mask slice around it).
                        # scale 1/8 applies the 1/sqrt(HD) softmax scale.
                        nc.scalar.activation(
                            ptp[:, :, off0:], pair[:, :, off0:], Exp,
                            scale=2.0 ** -3,
                        )
                        if t0 >= 0:
                            # diag-block triangle mask on the (otherwise
                            # idle) Pool engine; DVE keeps the normalize path
                            nc.gpsimd.tensor_mul(
                                ptp[:, 0, off0 : off0 + P],
                                ptp[:, 0, off0 : off0 + P],
                                mk_t,
                            )
                            nc.gpsimd.tensor_mul(
                                ptp[:, 1, off1 : off1 + P],
                                ptp[:, 1, off1 : off1 + P],
                                mk_t,
                            )
                        if len(pending) >= lag:
                            flush_one()

                        def pv_fn(po=po, ki0=ki0, ki1=ki1, off0=off0,
                                  off1=off1, ptp=ptp, h=h, nk=nk):
                            for i, (ki, off) in enumerate(
                                ((ki0, off0), (ki1, off1))
                            ):
                                kc, kb = divmod(ki, KPQ)
                                nc.tensor.matmul(
                                    po[:, off:],
                                    lhsT=vAs[kc][:, kb, h, :],
                                    rhs=ptp[:, i, off:],
                                    start=(ki == 0),
                                    stop=(ki == nk - 1),
                                    skip_group_check=True,
                                )
                        norm_fn = None
                        if pi == nk // 2 - 1:
                            def norm_fn(j=j, h=h, hb=hb, ho=ho, po=po):
                                # denominator reciprocal as exp(-ln(d)) on
                                # ACT (Ln and Exp share one activation table
                                # so no table reloads; DVE's RECIPROCAL is
                                # 3.4us per head and blocks the po bank),
                                # then one DVE multiply straight into the
                                # bf16 attn-out tile.
                                rl = spool.tile([HD, QC], FP32, tag="rl",
                                                name="rl")
                                nc.scalar.activation(rl, po[HD : 2 * HD, :], Ln)
                                rr = spool.tile([HD, QC], FP32, tag="rr",
                                                name="rr")
                                nc.scalar.activation(rr, rl, Exp, scale=-1.0)
                                nc.vector.tensor_mul(
                                    aTs[j][ho : ho + HD, hb, :],
                                    po[0:HD, :],
                                    rr,
                                )
                        pending.append((pv_fn, norm_fn))
                        # two fillers per pair keeps the PE ahead of the
                        # exp stream (chunk 0's first pairs stay at 1: its
                        # fillers' xa(1) half rides the sync DMA queue
                        # behind the weights and lands mid-chunk)
                        pump(2 if (j > 0 or pi >= 2) else 1)

                while pending:
                    flush_one()
                del qT_of[j]

                if j + 1 < NQC:
                    # chunk 1's output projection is deferred to chunk 3:
                    # chunk 2 is PE-bound (its own attention + chunk-3 QKV
                    # fillers) while chunk 3 is exp/ACT-bound with PE slack.
                    if j == 0:
                        fillers.append(outproj_gen(0))
                    elif j == 2:
                        fillers.append(outproj_gen(1))
                        fillers.append(outproj_gen(2))
                    pump_gen(gnext)
                else:
                    # last chunk: drain leftovers, then run the final output
                    # projection with the ib=0..1 accumulations pre-started in
                    # borrowed PSUM banks so the PE overlaps the last head's
                    # normalize chain; ib=2..3 (heads 4-7) finishes after it.
                    # (a partial drain here deadlocks: held-back outproj
                    # generators' psA tiles circularly wait on the borrowed
                    # final-projection banks)
                    while fillers:
                        pump(1)
                    groups = [(sb, nh) for sb in range(KPQ) for nh in range(2)]
                    pps = []
                    fs = None
                    for g, (sb, nh) in enumerate(groups):
                        if g < 4:
                            if g % 2 == 0:
                                fs = psS.tile(
                                    [P, 2, 512], FP32, tag="ps", name=f"fppS{g // 2}"
                                )
                            pp = fs[:, g % 2, :]
                        elif g < 6:
                            pp = psA.tile([P, 512], FP32, tag="psa", name=f"fpp{g}")
                        else:
                            pp = psO.tile([P, QC], FP32, tag="po", name=f"fpp{g}")
                        pps.append(pp)
                        # ib 0..2 (heads 0-5) prestarts while the last heads'
                        # normalize chains complete; only ib=3 (heads 6/7)
                        # remains gated on the final normalize
                        for ib in range(3):
                            nc.tensor.matmul(
                                pp,
                                lhsT=aTs[j][:, ib, sb * P : (sb + 1) * P],
                                rhs=wp_t[:, ib, nh * 512 : (nh + 1) * 512],
                                start=(ib == 0),
                                stop=False,
                                skip_group_check=True,
                            )
                    # finish in reverse group order: the earliest-finished
                    # stores go on the SWDGE queue (whose final drain gates
                    # the epilogue), the last ones on the faster sync queue
                    for g in reversed(range(len(groups))):
                        sb, nh = groups[g]
                        nc.tensor.matmul(
                            pps[g],
                            lhsT=aTs[j][:, 3, sb * P : (sb + 1) * P],
                            rhs=wp_t[:, 3, nh * 512 : (nh + 1) * 512],
                            start=False,
                            stop=True,
                            skip_group_check=True,
                        )
                        if g % 2 == 0:
                            emit_out_store(
                                j, sb, pps[g], pps[g + 1],
                                on_act=False, on_swdge=(g >= 4),
                                split=True,
                            )

            while fillers:
                pump(1)


def build_program(seq=S, split=True):
    nc = bass.Bass("TRN2", target_bir_lowering=False, debug=False, num_devices=N_CORES)
    io = {
        "xt": nc.dram_tensor("xt", [P, (seq // QC) * 8 * QC], BF, kind="ExternalInput").ap(),
        "wq": nc.dram_tensor("wq", [P, 8 * DHC], BF, kind="ExternalInput").ap(),
        "wk": nc.dram_tensor("wk", [P, 8 * DHC], BF, kind="ExternalInput").ap(),
        "wv": nc.dram_tensor("wv", [P, 8 * DHC], BF, kind="ExternalInput").ap(),
        "wp": nc.dram_tensor("wp", [P, 4 * D], BF, kind="ExternalInput").ap(),
        "bq": nc.dram_tensor("bq", [P, 4], FP32, kind="ExternalInput").ap(),
        "bk": nc.dram_tensor("bk", [P, 4], FP32, kind="ExternalInput").ap(),
        "bv": nc.dram_tensor("bv", [P, DHC], FP32, kind="ExternalInput").ap(),
        "masks": nc.dram_tensor("masks", [P, P], BF, kind="ExternalInput").ap(),
        "out": nc.dram_tensor("out", [seq, D], BF, kind="ExternalOutput").ap(),
    }
    build_attention(nc, io, seq=seq)
    if split:
        split_excess_waits(nc)
    return nc


def make_masks():
    kk = np.arange(P)[:, None]
    qq = np.arange(P)[None, :]
    return np.ascontiguousarray((kk <= qq).astype(NPBF))


def blk_w(w):
    """(K, N) -> [128, (K//128)*N] with row ki holding all (ko, n) blocks."""
    k, n = w.shape
    return np.ascontiguousarray(
        w.reshape(k // P, P, n).transpose(1, 0, 2).reshape(P, (k // P) * n)
    )


def blk_x(xb):
    """x (S, D) -> chunk-major blocked x^T: [128, NQC*8*QC]."""
    seq = xb.shape[0]
    a = xb.T.reshape(8, P, seq)  # [ko, ki, s]
    b = a.transpose(1, 0, 2).reshape(P, 8, seq // QC, QC).transpose(0, 2, 1, 3)
    return np.ascontiguousarray(b.reshape(P, (seq // QC) * 8 * QC))


def shard_inputs(x, Wq, bq, Wk, bk, Wv, bv, Wp, bp, seq=S):
    masks = make_masks()
    in_maps = []
    xcache = {}
    for c in range(N_CORES):
        b, hg = c // 2, c % 2
        cols = slice(hg * DHC, (hg + 1) * DHC)
        bqc = np.ascontiguousarray(bq[cols].reshape(4, P).T)
        bkc = np.ascontiguousarray(bk[cols].reshape(4, P).T)
        bvc = np.ascontiguousarray(np.tile(bv[cols][None, :], (P, 1)))
        if b not in xcache:
            xcache[b] = blk_x(x[b]).astype(NPBF)
        in_maps.append(
            {
                "xt": xcache[b],
                "wq": blk_w(Wq[:, cols]).astype(NPBF),
                "wk": blk_w(Wk[:, cols]).astype(NPBF),
                "wv": blk_w(Wv[:, cols]).astype(NPBF),
                "wp": blk_w(Wp[cols, :]).astype(NPBF),
                "bq": bqc,
                "bk": bkc,
                "bv": bvc,
                "masks": masks,
            }
        )
    return in_maps


_NC_CACHE = {}


def _get_nc(seq=S):
    if seq not in _NC_CACHE:
        _NC_CACHE[seq] = build_program(seq)
    return _NC_CACHE[seq]


def kernel(x, Wq, bq, Wk, bk, Wv, bv, Wp, bp, **run_kwargs):
    from concourse.bass_utils import run_bass_kernel_spmd

    x = np.asarray(x, np.float32)
    Wq, Wk, Wv, Wp = (np.asarray(a, np.float32) for a in (Wq, Wk, Wv, Wp))
    bq, bk, bv, bp = (np.asarray(a, np.float32) for a in (bq, bk, bv, bp))

    nc = _get_nc()
    in_maps = shard_inputs(x, Wq, bq, Wk, bk, Wv, bv, Wp, bp)
    res = run_bass_kernel_spmd(nc, in_maps, core_ids=list(range(N_CORES)), **run_kwargs)
    parts = [np.asarray(res.results[c]["out"], np.float32) for c in range(N_CORES)]
    out = np.empty((B, S, D), np.float32)
    for b in range(B):
        out[b] = parts[2 * b] + parts[2 * b + 1] + bp
    kernel.last_results = res
    return out
